# revision 29
# baseline (speedup 1.0000x reference)
"""Trainium2 Bass kernel for nn_Attention_54030688584207.

Single-head attention block:
    h = LN(x^T) ; qkv = h @ W^T + b ; S = q k^T / sqrt(N) + position
    out = softmax(S) @ v, returned as [B, C, N].

Sharding: 8 cores = 4 batches x 2 query-halves, no collectives. Each core
receives its batch's x rotated so its own 1024 query tokens come first and
computes q for its half plus full K/V for the batch, then scores/softmax/PV
for its 1024 query rows.

All large matmuls run as fp8e4(e4m3) DoubleRow 3-term hi/lo splits:
    A @ B ~= Ah Bh + Al Bh + Ah Bl       (ll term dropped, ~0.07%/elem)
Each DoubleRow instruction contracts K=256 (two 128-chunks packed in the
free dim) at 0.5 cycles per output column, i.e. 4x the bf16 FLOP rate, so
the 3-term split costs 0.75x of bf16 with ~bf16 accuracy. Operands are kept
at std~1 so the lo plane stays clear of the fp8 subnormal floor: W ships as
32*W^T, x ships premultiplied by rstd (LN fold), and the q/k/v epilogue
rescales by 1/32 while splitting.

LayerNorm statistics, the W column sums, and the softmax max-shift are
folded on the host:
    qkv[d,t] = (G[d,t] + (-mu*rstd)[t]*wsum[d]) / 32,  G = (32 W'^T)(x rstd)
    position ships as (pos[i,j] - rowmax_j(pos[i,:]) - 1.5)/SCALE in bf16,
so exp(SCALE*psum) is range-safe for fp8 (max ~80 < 240) and the per-query
shift cancels between the PV numerator and the row-sum denominator.

Per-tile epilogues are spread across the non-PE engines (val on DVE, hi
cast on Act, lo subtract on Pool; exp on Act, position add on DVE) so the
tensor engine stream is the only critical path. Row sums accumulate in one
PSUM bank via 1-wide DoubleRow matmuls on the es hi/lo tiles.

Device layouts (per core):
    x_hi/x_lo   [C, N] f8      (x*rstd, token-rotated, hi/lo split)
    w_hi/w_lo   [C, 3C] f8     (32*W'^T, hi/lo split)
    nmr_b       [128, N] bf16  (-mu*rstd, replicated rows)
    nmr_col     [128, NJT] f32 (-mu*rstd per token tile column)
    wsum        [3C] f32       (column sums of effective 32*W')
    wvs_b       [128, C] bf16  (v-part wsum, replicated rows)
    pos_t       [N, MY] bf16   ((pos - m_i)/SCALE, keys in local order)
    out         [MY, C] bf16   out[i, c] (host casts f32 + transposes)
"""

import os
import sys

for _p in ("/opt/trn_rl_repo",):
    if _p not in sys.path and os.path.isdir(_p):
        sys.path.insert(0, _p)

import numpy as np
import ml_dtypes

import concourse.bass as bass
import concourse.tile as tile
from concourse import bacc, mybir
from concourse.bass import ts, ds
from concourse.bass_utils import run_bass_kernel_spmd

FP = mybir.dt.float32
BF = mybir.dt.bfloat16
F8 = mybir.dt.float8e4
DR = mybir.MatmulPerfMode.DoubleRow
AF = mybir.ActivationFunctionType
MULT = mybir.AluOpType.mult
ADD = mybir.AluOpType.add
SUB = mybir.AluOpType.subtract

B = 4
C = 1024
N = 2048
MY = 1024  # query rows per core
D3 = 3 * C
NCH = C // 128   # 8 channel chunks
NCP = NCH // 2   # 4 channel chunk pairs
NJT = N // 128   # 16 key tiles
NJP = NJT // 2   # 8 key tile pairs
NIB = MY // 128  # 8 query blocks
LN_EPS = 1e-5
SCALE = 1.0 / np.sqrt(N)
WS = 32.0        # host weight pre-scale
M_SHIFT = -1.5   # softmax shift: m_i = rowmax + M_SHIFT, so P_max ~ e^(1.5+d)
                 # stays ~5..80 - high in fp8 range (floor noise) yet clip-safe


def build_kernel(rep=1, qk_bias=False, v_bias=False):
    nc = bacc.Bacc("TRN2", target_bir_lowering=False, debug=False, num_devices=8)
    xh_ext = nc.declare_dram_parameter("x_hi", [C, N], F8, isOutput=False)
    xl_ext = nc.declare_dram_parameter("x_lo", [C, N], F8, isOutput=False)
    wh_ext = nc.declare_dram_parameter("w_hi", [C, D3], F8, isOutput=False)
    wl_ext = nc.declare_dram_parameter("w_lo", [C, D3], F8, isOutput=False)
    nmrb_ext = nc.declare_dram_parameter("nmr_b", [128, N], F8, isOutput=False)
    nmrc_ext = nc.declare_dram_parameter("nmr_col", [128, NJT], FP, isOutput=False)
    ws_ext = nc.declare_dram_parameter("wsum", [128, 24], FP, isOutput=False)
    wvs_ext = nc.declare_dram_parameter("wvs_b", [128, C], F8, isOutput=False)
    b_ext = nc.declare_dram_parameter("bias", [128, 24], FP, isOutput=False)
    pos_ext = nc.declare_dram_parameter("pos_t", [N, MY], BF, isOutput=False)
    out_ext = nc.declare_dram_parameter("out", [MY, C], BF, isOutput=True)

    xh_r = xh_ext.ap().rearrange("(a p) n -> p a n", p=128)    # [128, 8, N]
    xl_r = xl_ext.ap().rearrange("(a p) n -> p a n", p=128)
    wh_r = wh_ext.ap().rearrange("(a p) d -> p a d", p=128)    # [128, 8, D3]
    wl_r = wl_ext.ap().rearrange("(a p) d -> p a d", p=128)

    with tile.TileContext(nc) as tc:
      for _r in range(rep):
        with (
            tc.tile_pool(name=f"res{_r}", bufs=1) as res,
            tc.tile_pool(name=f"statb{_r}", bufs=2) as statb,
            tc.tile_pool(name=f"pospool{_r}", bufs=3) as pospool,
            tc.tile_pool(name=f"valp{_r}", bufs=6) as valp,
            tc.tile_pool(name=f"rows{_r}", bufs=1) as rows,
            tc.tile_pool(name=f"psum{_r}", bufs=1, space="PSUM") as psum,
        ):
            # ---- resident tiles ----
            xh = res.tile([128, NCH, N], F8, tag="bigh", name="xh")
            xl = res.tile([128, NCH, N], F8, tag="bigl", name="xl")
            wqh = res.tile([128, NCH, 2 * C], F8, tag="wqh", name="wqh")
            wql = res.tile([128, NCH, 2 * C], F8, tag="wql", name="wql")
            wvh = res.tile([128, NCH, C], F8, tag="wvh", name="wvh")
            wvl = res.tile([128, NCH, C], F8, tag="wvl", name="wvl")
            qsh = res.tile([128, NCH, MY], F8, tag="qsh", name="qsh")
            qsl = res.tile([128, NCH, MY], F8, tag="qsl", name="qsl")
            ksh = res.tile([128, NCH, N], F8, tag="ksh", name="ksh")
            ksl = res.tile([128, NCH, N], F8, tag="ksl", name="ksl")
            vsh = res.tile([128, NJT, C], F8, tag="vsh", name="vsh")
            vsl = res.tile([128, NJT, C], F8, tag="vsl", name="vsl")

            ones2 = rows.tile([128, 2, 1], F8, tag="ones2", name="ones2")
            nc.vector.memset(ones2[:], 1.0)
            warm = rows.tile([128, 64], BF, tag="warm", name="warm")
            nc.vector.memset(warm[:], 0.0)

            nmr_b = rows.tile([128, N], F8, tag="nmrb", name="nmr_b")
            nmr_col = rows.tile([128, NJT], FP, tag="nmrc", name="nmr_col")
            wvsum_b = rows.tile([128, C], F8, tag="wvsb", name="wvsum_b")
            wsum_sb = rows.tile([128, 24], FP, tag="wsum", name="wsum_sb")
            bias_sb = rows.tile([128, 24], FP, tag="bias", name="bias_sb")

            # ---- input DMAs ----
            # One shared DMA device round-robins the three queues (SP / Act /
            # Pool), so spreading consumption-consecutive tensors across the
            # queues yields arrival in consumption order: xh0, wqh0, wql0,
            # xl0, wqh1, wql1, xh1, ... Stats ride the Act queue early enough
            # to unblock the first epilogues before PSUM pressure builds.
            nc.sync.dma_start(xh[:, :, ts(0, 512)], xh_r[:, :, ts(0, 512)])
            nc.sync.dma_start(xl[:, :, ts(0, 512)], xl_r[:, :, ts(0, 512)])
            nc.sync.dma_start(xh[:, :, ts(1, 512)], xh_r[:, :, ts(1, 512)])
            nc.sync.dma_start(xl[:, :, ts(1, 512)], xl_r[:, :, ts(1, 512)])
            nc.scalar.dma_start(wqh[:, :, ts(0, 512)], wh_r[:, :, ts(0, 512)])
            nc.scalar.dma_start(wqh[:, :, ts(1, 512)], wh_r[:, :, ts(1, 512)])
            nc.scalar.dma_start(nmr_b[:], nmrb_ext.ap())
            nc.scalar.dma_start(wsum_sb[:], ws_ext.ap())
            nc.scalar.dma_start(bias_sb[:], b_ext.ap())
            nc.scalar.dma_start(nmr_col[:], nmrc_ext.ap())
            nc.gpsimd.dma_start(wql[:, :, ts(0, 512)],
                                wl_r[:, :, ts(0, 512)])
            nc.gpsimd.dma_start(wql[:, :, ts(1, 512)],
                                wl_r[:, :, ts(1, 512)])

            # ---- PE ramp warm-up: burn the p-state window during DMA ----
            ps_w = psum.tile([128, 512], FP, tag="w", bufs=7, name="ps_w")
            ones_col = rows.tile([128, 1], BF, tag="onesc", name="ones_col")
            nc.vector.memset(ones_col[:], 0.0)
            for _ in range(120):
                nc.tensor.matmul(ps_w[0:1, ds(0, 64)], ones_col[:], warm[:],
                                 start=True, stop=True)

            # ---- 3-term DoubleRow contraction helper ----
            def mm3(ps, lh, ll, rh, rl, lslice, rslice, extra=0):
                """ps += (lh+ll).T (rh+rl) over all NCH chunks, 3 terms.
                lh/ll, rh/rl: [128, NCH, *] tiles; lslice/rslice: free slices.
                extra: count of further matmuls accumulating into ps after
                these (controls stop flag)."""
                k = 0
                for term in range(3):
                    lt = lh if term != 1 else ll
                    rt = rh if term != 2 else rl
                    for p in range(NCP):
                        nc.tensor.matmul(
                            ps, lt[:, ds(2 * p, 2), lslice],
                            rt[:, ds(2 * p, 2), rslice],
                            start=(k == 0), stop=(extra == 0 and k == 3 * NCP - 1),
                            perf_mode=DR)
                        k += 1

            # ---- q/k/v epilogue: val (DVE) -> hi (Act) -> lo (Pool) ----
            def qkv_epilogue(ps, dt, t, hi_dst, lo_dst, is_v=False, jt=None, alt=False):
                val = valp.tile([128, 512], BF, tag="val", name=f"val_{dt}_{t}")
                if is_v:
                    nc.vector.scalar_tensor_tensor(
                        val[:], wvsum_b[:, ts(t, 512)], nmr_col[:, jt:jt + 1],
                        ps, op0=MULT, op1=ADD)
                else:
                    nc.vector.scalar_tensor_tensor(
                        val[:], nmr_b[:, ts(t, 512)], wsum_sb[:, dt:dt + 1],
                        ps, op0=MULT, op1=ADD)
                if (qk_bias and not is_v) or (v_bias and is_v):
                    # bias ships pre-scaled by 32 to match val's scale
                    if is_v:
                        # v bias varies along free dim; add via broadcast row
                        nc.vector.tensor_add(val[:], val[:],
                                             bias_v_b[:, ts(t, 512)])
                    else:
                        nc.vector.tensor_scalar_add(val[:], val[:],
                                                    bias_sb[:, dt:dt + 1])
                nc.scalar.mul(hi_dst, val[:], 1.0 / WS)
                if alt:
                    # Pool path: 2 ops, keeps the DVE under the PE tile rate
                    t32 = valp.tile([128, 512], BF, tag="t32",
                                    name=f"t32_{dt}_{t}")
                    nc.gpsimd.tensor_scalar_mul(t32[:], val[:], 1.0 / WS)
                    nc.gpsimd.tensor_sub(lo_dst, t32[:], hi_dst)
                else:
                    nc.vector.scalar_tensor_tensor(
                        lo_dst, val[:], 1.0 / WS, hi_dst, op0=MULT, op1=SUB)

            if v_bias:
                # bias_sb[:, 16:24] holds the v bias as [p, a] (d = a*128+p);
                # the v epilogue needs it along the free (c) dim, replicated
                # over token partitions: bounce through DRAM to transpose.
                bias_v_b = rows.tile([128, C], FP, tag="bvb", name="bias_v_b")
                bvd = nc.declare_dram_parameter("bias_vd", [1, C], FP,
                                                isOutput=True)
                nc.gpsimd.dma_start(
                    bvd.ap().rearrange("o (a p) -> (o p) a", p=128),
                    bias_sb[:, ds(16, 8)])
                bvrow = statb.tile([1, C], FP, tag="bvrow", bufs=1, name="bvrow")
                nc.gpsimd.dma_start(bvrow[:], bvd.ap())
                nc.gpsimd.partition_broadcast(bias_v_b[:], bvrow[:])

            # ---- Phase B1: q^T and k^T ----
            # q: dt 0..7 (d-slices of q), t 0..1 ; k: dt 8..15, t 0..3.
            # q-part first (w chunks 0-1), k-part after (chunks 2-3), each
            # t-outer, matching DMA arrival. Tiles run in groups of 4 with
            # term-sliced emission (all hh, then lh, then hl) so the wql/xl
            # DMAs get 1.7-3.4us of in-group slack.
            b1_tiles = ([(dt, t) for t in range(2) for dt in range(8)]
                        + [(dt, t) for t in range(4) for dt in range(8, 16)])
            for g in range(0, len(b1_tiles), 4):
                if g == 8:
                    # k-part weights + remaining x chunks: emitted here so
                    # their issue slots behind the q-part traffic
                    nc.scalar.dma_start(wqh[:, :, ts(2, 512)],
                                        wh_r[:, :, ts(2, 512)])
                    nc.gpsimd.dma_start(wql[:, :, ts(2, 512)],
                                        wl_r[:, :, ts(2, 512)])
                    nc.scalar.dma_start(wqh[:, :, ts(3, 512)],
                                        wh_r[:, :, ts(3, 512)])
                    nc.gpsimd.dma_start(wql[:, :, ts(3, 512)],
                                        wl_r[:, :, ts(3, 512)])
                    for t in range(2, 4):
                        nc.sync.dma_start(xh[:, :, ts(t, 512)],
                                          xh_r[:, :, ts(t, 512)])
                        nc.sync.dma_start(xl[:, :, ts(t, 512)],
                                          xl_r[:, :, ts(t, 512)])
                elif g == 24:
                    nc.scalar.dma_start(wvh[:], wh_r[:, :, ds(2 * C, C)])
                    nc.gpsimd.dma_start(wvl[:], wl_r[:, :, ds(2 * C, C)])
                    nc.scalar.dma_start(wvsum_b[:], wvs_ext.ap())
                group = b1_tiles[g:g + 4]
                pss = {}
                for dt, t in group:
                    pss[(dt, t)] = psum.tile([128, 512], FP, tag="w", bufs=7,
                                             name=f"qk_{dt}_{t}")
                for term in range(3):
                    lt = wqh if term != 1 else wql
                    rt = xh if term != 2 else xl
                    for dt, t in group:
                        for p in range(NCP):
                            nc.tensor.matmul(
                                pss[(dt, t)][:],
                                lt[:, ds(2 * p, 2), ds(dt * 128, 128)],
                                rt[:, ds(2 * p, 2), ts(t, 512)],
                                start=(term == 0 and p == 0),
                                stop=(term == 2 and p == NCP - 1),
                                perf_mode=DR)
                for gi, (dt, t) in enumerate(group):
                    if dt < 8:
                        hi = qsh[:, dt, ts(t, 512)]
                        lo = qsl[:, dt, ts(t, 512)]
                    else:
                        hi = ksh[:, dt - 8, ts(t, 512)]
                        lo = ksl[:, dt - 8, ts(t, 512)]
                    qkv_epilogue(pss[(dt, t)][:], dt, t, hi, lo,
                                 alt=(gi % 2 == 1))

            # ---- Phase B2: v (x stationary) ----
            for jt in range(NJT):
                for cc in range(2):
                    ps = psum.tile([128, 512], FP, tag="w", bufs=7,
                                   name=f"v_{jt}_{cc}")
                    mm3(ps[:], xh, xl, wvh, wvl, ts(jt, 128), ts(cc, 512))
                    qkv_epilogue(ps[:], 16 + jt, cc, vsh[:, jt, ts(cc, 512)],
                                 vsl[:, jt, ts(cc, 512)], is_v=True, jt=jt,
                                 alt=(cc == 1))

            # ---- Phase C: S^T = k^T.T q^T (+pos, exp) -> es hi/lo ----
            esh = res.tile([128, NJT, MY], F8, tag="bigh", name="esh")
            esl = res.tile([128, NJT, MY], F8, tag="bigl", name="esl")
            ps_sums = psum.tile([128, NIB], FP, tag="sums", bufs=1,
                                name="ps_sums")

            def rowsums(jp, first, last):
                # ps_sums[:, i] += sum over j-pair jp of es hi+lo rows
                for i in range(NIB):
                    nc.tensor.matmul(
                        ps_sums[:, i:i + 1], esh[:, ds(2 * jp, 2), ts(i, 128)],
                        ones2[:], start=(first and i == 0), stop=False,
                        perf_mode=DR)
                for i in range(NIB):
                    nc.tensor.matmul(
                        ps_sums[:, i:i + 1], esl[:, ds(2 * jp, 2), ts(i, 128)],
                        ones2[:], start=False, stop=(last and i == NIB - 1),
                        perf_mode=DR)

            for j in range(NJT):
                pos_tile = pospool.tile([128, MY], BF, tag="pos")
                nc.scalar.dma_start(pos_tile[:], pos_ext[ts(j, 128), :])
                pss = [psum.tile([128, 512], FP, tag="w", bufs=7,
                                 name=f"s_{j}_{ih}") for ih in range(2)]
                for ih in range(2):
                    mm3(pss[ih][:], ksh, ksl, qsh, qsl, ts(j, 128),
                        ts(ih, 512))
                if j >= 3 and j % 2 == 1:
                    # pair (j-3, j-2): two tiles of slack vs the Pool lo-sub
                    rowsums((j - 3) // 2, first=(j == 3), last=False)
                esvs = []
                for ih in range(2):
                    nc.vector.tensor_add(pss[ih][:], pss[ih][:],
                                         pos_tile[:, ts(ih, 512)])
                for ih in range(2):
                    esv = valp.tile([128, 512], BF, tag="esv",
                                    name=f"esv_{j}_{ih}")
                    nc.scalar.activation(esv[:], pss[ih][:], AF.Exp,
                                         scale=SCALE)
                    esvs.append(esv)
                for ih in range(2):
                    if ih == 0 or j == NJT - 1:
                        nc.scalar.copy(esh[:, j, ts(ih, 512)], esvs[ih][:])
                    else:
                        nc.vector.tensor_copy(esh[:, j, ts(ih, 512)],
                                              esvs[ih][:])
                    if j == NJT - 1:
                        # last tile: phase D waits on these; DVE is faster
                        nc.vector.tensor_sub(esl[:, j, ts(ih, 512)],
                                             esvs[ih][:],
                                             esh[:, j, ts(ih, 512)])
                    else:
                        nc.gpsimd.tensor_sub(esl[:, j, ts(ih, 512)],
                                             esvs[ih][:],
                                             esh[:, j, ts(ih, 512)])

            # ---- Phase D: out[i, c] = (P^T)^T v / rowsum ----
            recips = rows.tile([128, NIB], FP, tag="recips", name="recips")

            def pv(ps, i, cc, tail_cb=None):
                # pairs 0..6 of every term first; the (14, 15) pair last so
                # the PE has ~2us of work before needing the final es tiles
                seq = ([(term, p) for term in range(3) for p in range(NJP - 1)]
                       + [(term, NJP - 1) for term in range(3)])
                for k, (term, p) in enumerate(seq):
                    if k == 3 * (NJP - 1) and tail_cb is not None:
                        tail_cb()
                    et = esh if term != 1 else esl
                    vt = vsh if term != 2 else vsl
                    nc.tensor.matmul(
                        ps, et[:, ds(2 * p, 2), ts(i, 128)],
                        vt[:, ds(2 * p, 2), ts(cc, 512)],
                        start=(k == 0), stop=(k == len(seq) - 1),
                        perf_mode=DR)

            for i in range(NIB):
                pso = [psum.tile([128, 512], FP, tag="w", bufs=7,
                                 name=f"o_{i}_{cc}") for cc in range(2)]
                if i == 0:
                    # last rowsum pair (14, 15) slots in after the pair-0..6
                    # PV matmuls; reciprocals follow
                    pv(pso[0][:], i, 0,
                       tail_cb=lambda: rowsums(NJP - 1, first=False, last=True))
                    nc.vector.reciprocal(recips[:], ps_sums[:])
                else:
                    pv(pso[0][:], i, 0)
                out_t = statb.tile([128, C], BF, tag="statb", bufs=2,
                                   name=f"out_t{i}")
                pv(pso[1][:], i, 1)
                nc.scalar.mul(out_t[:, ts(0, 512)], pso[0][:],
                              recips[:, i:i + 1])
                nc.sync.dma_start(out_ext[ts(i, 128), ts(0, 512)],
                                  out_t[:, ts(0, 512)])
                nc.scalar.mul(out_t[:, ts(1, 512)], pso[1][:],
                              recips[:, i:i + 1])
                nc.sync.dma_start(out_ext[ts(i, 128), ts(1, 512)],
                                  out_t[:, ts(1, 512)])

    nc.compile()
    return nc


_NC_CACHE = {}


def _get_nc(qk_bias, v_bias):
    key = (qk_bias, v_bias)
    if key not in _NC_CACHE:
        _NC_CACHE[key] = build_kernel(qk_bias=qk_bias, v_bias=v_bias)
    return _NC_CACHE[key]


def _split8(a):
    hi32 = np.clip(a, -240, 240).astype(ml_dtypes.float8_e4m3)
    lo = (a - hi32.astype(np.float32)).astype(ml_dtypes.float8_e4m3)
    return hi32, lo


def prep_in_maps(x, position, ln_gamma, ln_beta, W_qkv, b_qkv):
    """Host-side sharding / layout prep. Returns in_maps for 8 cores."""
    x = np.asarray(x, dtype=np.float32)
    position = np.asarray(position, dtype=np.float32)
    ln_gamma = np.asarray(ln_gamma, dtype=np.float32)
    ln_beta = np.asarray(ln_beta, dtype=np.float32)
    W_qkv = np.asarray(W_qkv, dtype=np.float32)
    b_qkv = np.asarray(b_qkv, dtype=np.float32)

    # Fold gamma into W columns, beta into bias. SCALE is applied at exp.
    # bias ships pre-scaled by WS to match the 32x val scale in the epilogue.
    Wp = W_qkv * ln_gamma[None, :]
    bp = (WS * (b_qkv + W_qkv @ ln_beta)).copy()
    Ws = np.ascontiguousarray(WS * Wp.T)          # [C, 3C]
    w_hi, w_lo = _split8(Ws)
    weff = w_hi.astype(np.float32) + w_lo.astype(np.float32)
    wsum = np.ascontiguousarray(weff.sum(axis=0), dtype=np.float32)
    wsum_2d = np.ascontiguousarray(wsum.reshape(24, 128).T)
    wvs_b = np.broadcast_to(np.clip(wsum[2 * C:], -240, 240).astype(
        ml_dtypes.float8_e4m3), (128, C)).copy()

    # position: per-query max-shift + 1/SCALE scaling, bf16
    m = position.max(axis=1) + M_SHIFT            # [N] per query i
    posp = (position - m[:, None]) / SCALE        # [i, j]

    in_maps = []
    for core in range(8):
        b, s = divmod(core, 2)
        xb = x[b]
        mu = xb.mean(axis=0)
        var = ((xb - mu) ** 2).mean(axis=0)
        rstd = 1.0 / np.sqrt(var + LN_EPS)
        if s == 1:
            xb = np.roll(xb, -MY, axis=1)
            mu = np.roll(mu, -MY)
            rstd = np.roll(rstd, -MY)
            pos_rot = np.roll(posp, -MY, axis=1)
        else:
            pos_rot = posp
        xr = xb * rstd[None, :]
        x_hi, x_lo = _split8(xr)
        nmr = np.clip(-mu * rstd, -240, 240).astype(ml_dtypes.float8_e4m3)
        nmr_b = np.broadcast_to(nmr, (128, N)).copy()
        nmr_col = np.ascontiguousarray(
            (-mu * rstd).reshape(NJT, 128).T, dtype=np.float32)
        pos_t = np.ascontiguousarray(
            pos_rot[s * MY:(s + 1) * MY, :].T).astype(ml_dtypes.bfloat16)
        in_maps.append({
            "x_hi": x_hi, "x_lo": x_lo,
            "w_hi": w_hi, "w_lo": w_lo,
            "nmr_b": nmr_b, "nmr_col": nmr_col,
            "wsum": wsum_2d, "wvs_b": wvs_b,
            "bias": np.ascontiguousarray(bp.reshape(24, 128).T),
            "pos_t": pos_t,
        })
    return in_maps


def kernel(x, position, ln_gamma, ln_beta, W_qkv, b_qkv):
    in_maps = prep_in_maps(x, position, ln_gamma, ln_beta, W_qkv, b_qkv)
    bp = in_maps[0]["bias"]  # [128, 24]: cols 0:16 are q,k; 16:24 are v
    nc = _get_nc(bool(np.abs(bp[:, :16]).max() > 0),
                 bool(np.abs(bp[:, 16:]).max() > 0))
    res = run_bass_kernel_spmd(nc, in_maps, core_ids=list(range(8)))
    out = np.empty((B, C, N), dtype=np.float32)
    for core in range(8):
        b, s = divmod(core, 2)
        out[b, :, s * MY:(s + 1) * MY] = res.results[core]["out"].astype(np.float32).T
    return out


# revision 36
# speedup vs baseline: 1.0105x; 1.0105x over previous
"""Trainium2 Bass kernel for nn_Attention_54030688584207.

Single-head attention block:
    h = LN(x^T) ; qkv = h @ W^T + b ; S = q k^T / sqrt(N) + position
    out = softmax(S) @ v, returned as [B, C, N].

Sharding: 8 cores = 4 batches x 2 query-halves, no collectives. Each core
receives its batch's x rotated so its own 1024 query tokens come first and
computes q for its half plus full K/V for the batch, then scores/softmax/PV
for its 1024 query rows.

All large matmuls run as fp8e4(e4m3) DoubleRow 3-term hi/lo splits:
    A @ B ~= Ah Bh + Al Bh + Ah Bl       (ll term dropped, ~0.07%/elem)
Each DoubleRow instruction contracts K=256 (two 128-chunks packed in the
free dim) at 0.5 cycles per output column, i.e. 4x the bf16 FLOP rate, so
the 3-term split costs 0.75x of bf16 with ~bf16 accuracy. Operands are kept
at std~1 so the lo plane stays clear of the fp8 subnormal floor: W ships as
32*W^T, x ships premultiplied by rstd (LN fold), and the q/k/v epilogue
rescales by 1/32 while splitting.

LayerNorm statistics, the W column sums, and the softmax max-shift are
folded on the host:
    qkv[d,t] = (G[d,t] + (-mu*rstd)[t]*wsum[d]) / 32,  G = (32 W'^T)(x rstd)
    position ships as (pos[i,j] - rowmax_j(pos[i,:]) - 1.5)/SCALE in bf16,
so exp(SCALE*psum) is range-safe for fp8 (max ~80 < 240) and the per-query
shift cancels between the PV numerator and the row-sum denominator.

Per-tile epilogues are spread across the non-PE engines (val on DVE, hi
cast on Act, lo subtract on Pool; exp on Act, position add on DVE) so the
tensor engine stream is the only critical path. Row sums accumulate in one
PSUM bank via 1-wide DoubleRow matmuls on the es hi/lo tiles.

Device layouts (per core):
    x_hi/x_lo   [C, N] f8      (x*rstd, token-rotated, hi/lo split)
    w_hi/w_lo   [C, 3C] f8     (32*W'^T, hi/lo split)
    nmr_b       [128, N] f8    (-mu*rstd, replicated rows)
    nmr_col     [128, NJT] f32 (-mu*rstd per token tile column)
    wsum        [128, 24] f32  (column sums of effective 32*W', [p, a])
    wvs_b       [128, C] f8    (v-part wsum, replicated rows)
    pos_t       [N, MY] bf16   ((pos - m_i)/SCALE, keys in local order)
    out         [MY, C] bf16   out[i, c] (host casts f32 + transposes)
"""

import os
import sys

for _p in ("/opt/trn_rl_repo",):
    if _p not in sys.path and os.path.isdir(_p):
        sys.path.insert(0, _p)

import numpy as np
import ml_dtypes

import concourse.bass as bass
import concourse.tile as tile
from concourse import bacc, mybir
from concourse.bass import ts, ds
from concourse.bass_utils import run_bass_kernel_spmd

FP = mybir.dt.float32
BF = mybir.dt.bfloat16
F8 = mybir.dt.float8e4
DR = mybir.MatmulPerfMode.DoubleRow
AF = mybir.ActivationFunctionType
MULT = mybir.AluOpType.mult
ADD = mybir.AluOpType.add
SUB = mybir.AluOpType.subtract

B = 4
C = 1024
N = 2048
MY = 1024  # query rows per core
D3 = 3 * C
NCH = C // 128   # 8 channel chunks
NCP = NCH // 2   # 4 channel chunk pairs
NJT = N // 128   # 16 key tiles
NJP = NJT // 2   # 8 key tile pairs
NIB = MY // 128  # 8 query blocks
LN_EPS = 1e-5
SCALE = 1.0 / np.sqrt(N)
WS = 32.0        # host weight pre-scale
M_SHIFT = -1.5   # softmax shift: m_i = rowmax + M_SHIFT, so P_max ~ e^(1.5+d)
                 # stays ~5..80 - high in fp8 range (floor noise) yet clip-safe


def build_kernel(rep=1, qk_bias=False, v_bias=False):
    nc = bacc.Bacc("TRN2", target_bir_lowering=False, debug=False, num_devices=8)
    xh_ext = nc.declare_dram_parameter("x_hi", [C, N], F8, isOutput=False)
    xl_ext = nc.declare_dram_parameter("x_lo", [C, N], F8, isOutput=False)
    wh_ext = nc.declare_dram_parameter("w_hi", [C, D3], F8, isOutput=False)
    wl_ext = nc.declare_dram_parameter("w_lo", [C, D3], F8, isOutput=False)
    nmrb_ext = nc.declare_dram_parameter("nmr_b", [128, N], F8, isOutput=False)
    nmrc_ext = nc.declare_dram_parameter("nmr_col", [128, NJT], FP, isOutput=False)
    ws_ext = nc.declare_dram_parameter("wsum", [128, 24], FP, isOutput=False)
    wvs_ext = nc.declare_dram_parameter("wvs_b", [128, C], F8, isOutput=False)
    b_ext = nc.declare_dram_parameter("bias", [128, 24], FP, isOutput=False)
    pos_ext = nc.declare_dram_parameter("pos_t", [N, MY], BF, isOutput=False)
    out_ext = nc.declare_dram_parameter("out", [MY, C], BF, isOutput=True)

    xh_r = xh_ext.ap().rearrange("(a p) n -> p a n", p=128)    # [128, 8, N]
    xl_r = xl_ext.ap().rearrange("(a p) n -> p a n", p=128)
    wh_r = wh_ext.ap().rearrange("(a p) d -> p a d", p=128)    # [128, 8, D3]
    wl_r = wl_ext.ap().rearrange("(a p) d -> p a d", p=128)

    with tile.TileContext(nc) as tc:
      for _r in range(rep):
        with (
            tc.tile_pool(name=f"res{_r}", bufs=1) as res,
            tc.tile_pool(name=f"statb{_r}", bufs=2) as statb,
            tc.tile_pool(name=f"pospool{_r}", bufs=3) as pospool,
            tc.tile_pool(name=f"valp{_r}", bufs=6) as valp,
            tc.tile_pool(name=f"rows{_r}", bufs=1) as rows,
            tc.tile_pool(name=f"psum{_r}", bufs=1, space="PSUM") as psum,
        ):
            # ---- resident tiles ----
            xh = res.tile([128, NCH, N], F8, tag="bigh", name="xh")
            xl = res.tile([128, NCH, N], F8, tag="bigl", name="xl")
            wqh = res.tile([128, NCH, 2 * C], F8, tag="wqh", name="wqh")
            wql = res.tile([128, NCH, 2 * C], F8, tag="wql", name="wql")
            wvh = res.tile([128, NCH, C], F8, tag="wvh", name="wvh")
            wvl = res.tile([128, NCH, C], F8, tag="wvl", name="wvl")
            qsh = res.tile([128, NCH, MY], F8, tag="qsh", name="qsh")
            qsl = res.tile([128, NCH, MY], F8, tag="qsl", name="qsl")
            ksh = res.tile([128, NCH, N], F8, tag="ksh", name="ksh")
            ksl = res.tile([128, NCH, N], F8, tag="ksl", name="ksl")
            vsh = res.tile([128, NJT, C], F8, tag="vsh", name="vsh")
            vsl = res.tile([128, NJT, C], F8, tag="vsl", name="vsl")

            ones2 = rows.tile([128, 2, 1], F8, tag="ones2", name="ones2")
            nc.vector.memset(ones2[:], 1.0)
            warm = rows.tile([128, 64], BF, tag="warm", name="warm")
            nc.vector.memset(warm[:], 0.0)

            nmr_b = rows.tile([128, N], F8, tag="nmrb", name="nmr_b")
            nmr_col = rows.tile([128, NJT], FP, tag="nmrc", name="nmr_col")
            wvsum_b = rows.tile([128, C], F8, tag="wvsb", name="wvsum_b")
            wsum_sb = rows.tile([128, 24], FP, tag="wsum", name="wsum_sb")
            bias_sb = rows.tile([128, 24], FP, tag="bias", name="bias_sb")

            # ---- input DMAs ----
            # One shared DMA device round-robins the three queues (SP / Act /
            # Pool), so spreading consumption-consecutive tensors across the
            # queues yields arrival in consumption order: xh0, wqh0, wql0,
            # xl0, wqh1, wql1, xh1, ... Stats ride the Act queue early enough
            # to unblock the first epilogues before PSUM pressure builds.
            nc.sync.dma_start(xh[:, :, ts(0, 512)], xh_r[:, :, ts(0, 512)])
            nc.sync.dma_start(xl[:, :, ts(0, 512)], xl_r[:, :, ts(0, 512)])
            nc.sync.dma_start(xh[:, :, ts(1, 512)], xh_r[:, :, ts(1, 512)])
            nc.sync.dma_start(xl[:, :, ts(1, 512)], xl_r[:, :, ts(1, 512)])
            nc.scalar.dma_start(wqh[:, :, ts(0, 512)], wh_r[:, :, ts(0, 512)])
            nc.scalar.dma_start(wqh[:, :, ts(1, 512)], wh_r[:, :, ts(1, 512)])
            nc.scalar.dma_start(wsum_sb[:], ws_ext.ap())
            nc.scalar.dma_start(bias_sb[:], b_ext.ap())
            nc.scalar.dma_start(nmr_col[:], nmrc_ext.ap())
            nc.gpsimd.dma_start(wql[:, :, ts(0, 512)],
                                wl_r[:, :, ts(0, 512)])
            nc.gpsimd.dma_start(nmr_b[:], nmrb_ext.ap())
            nc.gpsimd.dma_start(wql[:, :, ts(1, 512)],
                                wl_r[:, :, ts(1, 512)])

            # ---- PE ramp warm-up: burn the p-state window during DMA ----
            ps_w = psum.tile([128, 512], FP, tag="w", bufs=7, name="ps_w")
            ones_col = rows.tile([128, 1], BF, tag="onesc", name="ones_col")
            nc.vector.memset(ones_col[:], 0.0)
            for _ in range(120):
                nc.tensor.matmul(ps_w[0:1, ds(0, 64)], ones_col[:], warm[:],
                                 start=True, stop=True)

            # ---- 3-term DoubleRow contraction helper ----
            def mm3(ps, lh, ll, rh, rl, lslice, rslice, extra=0):
                """ps += (lh+ll).T (rh+rl) over all NCH chunks, 3 terms.
                lh/ll, rh/rl: [128, NCH, *] tiles; lslice/rslice: free slices.
                extra: count of further matmuls accumulating into ps after
                these (controls stop flag)."""
                k = 0
                for term in range(3):
                    lt = lh if term != 1 else ll
                    rt = rh if term != 2 else rl
                    for p in range(NCP):
                        nc.tensor.matmul(
                            ps, lt[:, ds(2 * p, 2), lslice],
                            rt[:, ds(2 * p, 2), rslice],
                            start=(k == 0), stop=(extra == 0 and k == 3 * NCP - 1),
                            perf_mode=DR)
                        k += 1

            # ---- q/k/v epilogue: val (DVE) -> hi (Act) -> lo (Pool) ----
            def qkv_epilogue(ps, dt, t, hi_dst, lo_dst, is_v=False, jt=None,
                             alt=False, defer=False):
                val = valp.tile([128, 512], BF, tag="val", name=f"val_{dt}_{t}")
                if defer:
                    # free the PSUM bank before nmr_b/wsum arrive: raw copy
                    # first, correction later from SBUF
                    vraw = valp.tile([128, 512], BF, tag="vraw",
                                     name=f"vraw_{dt}_{t}")
                    nc.vector.tensor_copy(vraw[:], ps)
                    nc.vector.scalar_tensor_tensor(
                        val[:], nmr_b[:, ts(t, 512)], wsum_sb[:, dt:dt + 1],
                        vraw[:], op0=MULT, op1=ADD)
                elif is_v:
                    nc.vector.scalar_tensor_tensor(
                        val[:], wvsum_b[:, ts(t, 512)], nmr_col[:, jt:jt + 1],
                        ps, op0=MULT, op1=ADD)
                else:
                    nc.vector.scalar_tensor_tensor(
                        val[:], nmr_b[:, ts(t, 512)], wsum_sb[:, dt:dt + 1],
                        ps, op0=MULT, op1=ADD)
                if (qk_bias and not is_v) or (v_bias and is_v):
                    # bias ships pre-scaled by 32 to match val's scale
                    if is_v:
                        # v bias varies along free dim; add via broadcast row
                        nc.vector.tensor_add(val[:], val[:],
                                             bias_v_b[:, ts(t, 512)])
                    else:
                        nc.vector.tensor_scalar_add(val[:], val[:],
                                                    bias_sb[:, dt:dt + 1])
                nc.scalar.mul(hi_dst, val[:], 1.0 / WS)
                if alt:
                    # Pool path: 2 ops, keeps the DVE under the PE tile rate
                    t32 = valp.tile([128, 512], BF, tag="t32",
                                    name=f"t32_{dt}_{t}")
                    nc.gpsimd.tensor_scalar_mul(t32[:], val[:], 1.0 / WS)
                    nc.gpsimd.tensor_sub(lo_dst, t32[:], hi_dst)
                else:
                    nc.vector.scalar_tensor_tensor(
                        lo_dst, val[:], 1.0 / WS, hi_dst, op0=MULT, op1=SUB)

            if v_bias:
                # bias_sb[:, 16:24] holds the v bias as [p, a] (d = a*128+p);
                # the v epilogue needs it along the free (c) dim, replicated
                # over token partitions: bounce through DRAM to transpose.
                bias_v_b = rows.tile([128, C], FP, tag="bvb", name="bias_v_b")
                bvd = nc.declare_dram_parameter("bias_vd", [1, C], FP,
                                                isOutput=True)
                nc.gpsimd.dma_start(
                    bvd.ap().rearrange("o (a p) -> (o p) a", p=128),
                    bias_sb[:, ds(16, 8)])
                bvrow = statb.tile([1, C], FP, tag="bvrow", bufs=1, name="bvrow")
                nc.gpsimd.dma_start(bvrow[:], bvd.ap())
                nc.gpsimd.partition_broadcast(bias_v_b[:], bvrow[:])

            # ---- Phase B1: q^T and k^T ----
            # q: dt 0..7 (d-slices of q), t 0..1 ; k: dt 8..15, t 0..3.
            # q-part first (w chunks 0-1), k-part after (chunks 2-3), each
            # t-outer, matching DMA arrival. Tiles run in groups of 4 with
            # term-sliced emission (all hh, then lh, then hl) so the wql/xl
            # DMAs get 1.7-3.4us of in-group slack.
            b1_tiles = ([(dt, t) for t in range(2) for dt in range(8)]
                        + [(dt, t) for t in range(4) for dt in range(8, 16)])
            for g in range(0, len(b1_tiles), 4):
                if g == 8:
                    # k-part weights + remaining x chunks: emitted here so
                    # their issue slots behind the q-part traffic
                    nc.scalar.dma_start(wqh[:, :, ts(2, 512)],
                                        wh_r[:, :, ts(2, 512)])
                    nc.gpsimd.dma_start(wql[:, :, ts(2, 512)],
                                        wl_r[:, :, ts(2, 512)])
                    nc.scalar.dma_start(wqh[:, :, ts(3, 512)],
                                        wh_r[:, :, ts(3, 512)])
                    nc.gpsimd.dma_start(wql[:, :, ts(3, 512)],
                                        wl_r[:, :, ts(3, 512)])
                    for t in range(2, 4):
                        nc.sync.dma_start(xh[:, :, ts(t, 512)],
                                          xh_r[:, :, ts(t, 512)])
                        nc.sync.dma_start(xl[:, :, ts(t, 512)],
                                          xl_r[:, :, ts(t, 512)])
                elif g == 24:
                    nc.scalar.dma_start(wvh[:], wh_r[:, :, ds(2 * C, C)])
                    nc.gpsimd.dma_start(wvl[:], wl_r[:, :, ds(2 * C, C)])
                    nc.scalar.dma_start(wvsum_b[:], wvs_ext.ap())
                group = b1_tiles[g:g + 4]
                pss = {}
                for dt, t in group:
                    pss[(dt, t)] = psum.tile([128, 512], FP, tag="w", bufs=7,
                                             name=f"qk_{dt}_{t}")
                for term in range(3):
                    lt = wqh if term != 1 else wql
                    rt = xh if term != 2 else xl
                    for dt, t in group:
                        for p in range(NCP):
                            nc.tensor.matmul(
                                pss[(dt, t)][:],
                                lt[:, ds(2 * p, 2), ds(dt * 128, 128)],
                                rt[:, ds(2 * p, 2), ts(t, 512)],
                                start=(term == 0 and p == 0),
                                stop=(term == 2 and p == NCP - 1),
                                perf_mode=DR)
                for gi, (dt, t) in enumerate(group):
                    if dt < 8:
                        hi = qsh[:, dt, ts(t, 512)]
                        lo = qsl[:, dt, ts(t, 512)]
                    else:
                        hi = ksh[:, dt - 8, ts(t, 512)]
                        lo = ksl[:, dt - 8, ts(t, 512)]
                    qkv_epilogue(pss[(dt, t)][:], dt, t, hi, lo,
                                 alt=(gi % 2 == 1))

            # ---- Phase B2: v (x stationary) ----
            for jt in range(NJT):
                for cc in range(2):
                    ps = psum.tile([128, 512], FP, tag="w", bufs=7,
                                   name=f"v_{jt}_{cc}")
                    mm3(ps[:], xh, xl, wvh, wvl, ts(jt, 128), ts(cc, 512))
                    qkv_epilogue(ps[:], 16 + jt, cc, vsh[:, jt, ts(cc, 512)],
                                 vsl[:, jt, ts(cc, 512)], is_v=True, jt=jt,
                                 alt=(cc == 1))

            # ---- Phase C: S^T = k^T.T q^T (+pos, exp) -> es hi/lo ----
            esh = res.tile([128, NJT, MY], F8, tag="bigh", name="esh")
            esl = res.tile([128, NJT, MY], F8, tag="bigl", name="esl")
            ps_sums = psum.tile([128, NIB], FP, tag="sums", bufs=1,
                                name="ps_sums")

            def rowsums(jp, first, last):
                # ps_sums[:, i] += sum over j-pair jp of es hi+lo rows
                for i in range(NIB):
                    nc.tensor.matmul(
                        ps_sums[:, i:i + 1], esh[:, ds(2 * jp, 2), ts(i, 128)],
                        ones2[:], start=(first and i == 0), stop=False,
                        perf_mode=DR)
                for i in range(NIB):
                    nc.tensor.matmul(
                        ps_sums[:, i:i + 1], esl[:, ds(2 * jp, 2), ts(i, 128)],
                        ones2[:], start=False, stop=(last and i == NIB - 1),
                        perf_mode=DR)

            for j in range(NJT):
                pos_tile = pospool.tile([128, MY], BF, tag="pos")
                nc.scalar.dma_start(pos_tile[:], pos_ext[ts(j, 128), :])
                pss = [psum.tile([128, 512], FP, tag="w", bufs=7,
                                 name=f"s_{j}_{ih}") for ih in range(2)]
                for ih in range(2):
                    mm3(pss[ih][:], ksh, ksl, qsh, qsl, ts(j, 128),
                        ts(ih, 512))
                if j >= 3 and j % 2 == 1:
                    # pair (j-3, j-2): two tiles of slack vs the Pool lo-sub
                    rowsums((j - 3) // 2, first=(j == 3), last=False)
                esvs = []
                for ih in range(2):
                    nc.vector.tensor_add(pss[ih][:], pss[ih][:],
                                         pos_tile[:, ts(ih, 512)])
                for ih in range(2):
                    esv = valp.tile([128, 512], BF, tag="esv",
                                    name=f"esv_{j}_{ih}")
                    nc.scalar.activation(esv[:], pss[ih][:], AF.Exp,
                                         scale=SCALE)
                    esvs.append(esv)
                for ih in range(2):
                    if ih == 0 or j == NJT - 1:
                        nc.scalar.copy(esh[:, j, ts(ih, 512)], esvs[ih][:])
                    else:
                        nc.vector.tensor_copy(esh[:, j, ts(ih, 512)],
                                              esvs[ih][:])
                    if j == NJT - 1:
                        # last tile: phase D waits on these; DVE is faster
                        nc.vector.tensor_sub(esl[:, j, ts(ih, 512)],
                                             esvs[ih][:],
                                             esh[:, j, ts(ih, 512)])
                    else:
                        nc.gpsimd.tensor_sub(esl[:, j, ts(ih, 512)],
                                             esvs[ih][:],
                                             esh[:, j, ts(ih, 512)])

            # ---- Phase D: out[i, c] = (P^T)^T v / rowsum ----
            recips = rows.tile([128, NIB], FP, tag="recips", name="recips")

            def pv(ps, i, cc, tail_cb=None):
                # pairs 0..6 of every term first; the (14, 15) pair last so
                # the PE has ~2us of work before needing the final es tiles
                seq = ([(term, p) for term in range(3) for p in range(NJP - 1)]
                       + [(term, NJP - 1) for term in range(3)])
                for k, (term, p) in enumerate(seq):
                    if k == 3 * (NJP - 1) and tail_cb is not None:
                        tail_cb()
                    et = esh if term != 1 else esl
                    vt = vsh if term != 2 else vsl
                    nc.tensor.matmul(
                        ps, et[:, ds(2 * p, 2), ts(i, 128)],
                        vt[:, ds(2 * p, 2), ts(cc, 512)],
                        start=(k == 0), stop=(k == len(seq) - 1),
                        perf_mode=DR)

            for i in range(NIB):
                pso = [psum.tile([128, 512], FP, tag="w", bufs=7,
                                 name=f"o_{i}_{cc}") for cc in range(2)]
                if i == 0:
                    # last rowsum pair (14, 15) slots in after the pair-0..6
                    # PV matmuls; reciprocals follow
                    pv(pso[0][:], i, 0,
                       tail_cb=lambda: rowsums(NJP - 1, first=False, last=True))
                    nc.vector.reciprocal(recips[:], ps_sums[:])
                else:
                    pv(pso[0][:], i, 0)
                out_t = statb.tile([128, C], BF, tag="statb", bufs=2,
                                   name=f"out_t{i}")
                pv(pso[1][:], i, 1)
                nc.scalar.mul(out_t[:, ts(0, 512)], pso[0][:],
                              recips[:, i:i + 1])
                nc.sync.dma_start(out_ext[ts(i, 128), ts(0, 512)],
                                  out_t[:, ts(0, 512)])
                nc.scalar.mul(out_t[:, ts(1, 512)], pso[1][:],
                              recips[:, i:i + 1])
                nc.sync.dma_start(out_ext[ts(i, 128), ts(1, 512)],
                                  out_t[:, ts(1, 512)])

    nc.compile()
    return nc


_NC_CACHE = {}


def _get_nc(qk_bias, v_bias):
    key = (qk_bias, v_bias)
    if key not in _NC_CACHE:
        _NC_CACHE[key] = build_kernel(qk_bias=qk_bias, v_bias=v_bias)
    return _NC_CACHE[key]


def _split8(a):
    hi32 = np.clip(a, -240, 240).astype(ml_dtypes.float8_e4m3)
    lo = (a - hi32.astype(np.float32)).astype(ml_dtypes.float8_e4m3)
    return hi32, lo


def prep_in_maps(x, position, ln_gamma, ln_beta, W_qkv, b_qkv):
    """Host-side sharding / layout prep. Returns in_maps for 8 cores."""
    x = np.asarray(x, dtype=np.float32)
    position = np.asarray(position, dtype=np.float32)
    ln_gamma = np.asarray(ln_gamma, dtype=np.float32)
    ln_beta = np.asarray(ln_beta, dtype=np.float32)
    W_qkv = np.asarray(W_qkv, dtype=np.float32)
    b_qkv = np.asarray(b_qkv, dtype=np.float32)

    # Fold gamma into W columns, beta into bias. SCALE is applied at exp.
    # bias ships pre-scaled by WS to match the 32x val scale in the epilogue.
    Wp = W_qkv * ln_gamma[None, :]
    bp = (WS * (b_qkv + W_qkv @ ln_beta)).copy()
    Ws = np.ascontiguousarray(WS * Wp.T)          # [C, 3C]
    w_hi, w_lo = _split8(Ws)
    weff = w_hi.astype(np.float32) + w_lo.astype(np.float32)
    wsum = np.ascontiguousarray(weff.sum(axis=0), dtype=np.float32)
    wsum_2d = np.ascontiguousarray(wsum.reshape(24, 128).T)
    wvs_b = np.broadcast_to(np.clip(wsum[2 * C:], -240, 240).astype(
        ml_dtypes.float8_e4m3), (128, C)).copy()

    # position: per-query max-shift + 1/SCALE scaling, bf16
    m = position.max(axis=1) + M_SHIFT            # [N] per query i
    posp = (position - m[:, None]) / SCALE        # [i, j]

    in_maps = []
    for core in range(8):
        b, s = divmod(core, 2)
        xb = x[b]
        mu = xb.mean(axis=0)
        var = ((xb - mu) ** 2).mean(axis=0)
        rstd = 1.0 / np.sqrt(var + LN_EPS)
        if s == 1:
            xb = np.roll(xb, -MY, axis=1)
            mu = np.roll(mu, -MY)
            rstd = np.roll(rstd, -MY)
            pos_rot = np.roll(posp, -MY, axis=1)
        else:
            pos_rot = posp
        xr = xb * rstd[None, :]
        x_hi, x_lo = _split8(xr)
        nmr = np.clip(-mu * rstd, -240, 240).astype(ml_dtypes.float8_e4m3)
        nmr_b = np.broadcast_to(nmr, (128, N)).copy()
        nmr_col = np.ascontiguousarray(
            (-mu * rstd).reshape(NJT, 128).T, dtype=np.float32)
        pos_t = np.ascontiguousarray(
            pos_rot[s * MY:(s + 1) * MY, :].T).astype(ml_dtypes.bfloat16)
        in_maps.append({
            "x_hi": x_hi, "x_lo": x_lo,
            "w_hi": w_hi, "w_lo": w_lo,
            "nmr_b": nmr_b, "nmr_col": nmr_col,
            "wsum": wsum_2d, "wvs_b": wvs_b,
            "bias": np.ascontiguousarray(bp.reshape(24, 128).T),
            "pos_t": pos_t,
        })
    return in_maps


def kernel(x, position, ln_gamma, ln_beta, W_qkv, b_qkv):
    in_maps = prep_in_maps(x, position, ln_gamma, ln_beta, W_qkv, b_qkv)
    bp = in_maps[0]["bias"]  # [128, 24]: cols 0:16 are q,k; 16:24 are v
    nc = _get_nc(bool(np.abs(bp[:, :16]).max() > 0),
                 bool(np.abs(bp[:, 16:]).max() > 0))
    res = run_bass_kernel_spmd(nc, in_maps, core_ids=list(range(8)))
    out = np.empty((B, C, N), dtype=np.float32)
    for core in range(8):
        b, s = divmod(core, 2)
        out[b, :, s * MY:(s + 1) * MY] = res.results[core]["out"].astype(np.float32).T
    return out


# revision 39
# speedup vs baseline: 1.0118x; 1.0013x over previous
"""Trainium2 Bass kernel for nn_Attention_54030688584207.

Single-head attention block:
    h = LN(x^T) ; qkv = h @ W^T + b ; S = q k^T / sqrt(N) + position
    out = softmax(S) @ v, returned as [B, C, N].

Sharding: 8 cores = 4 batches x 2 query-halves, no collectives. Each core
receives its batch's x rotated so its own 1024 query tokens come first and
computes q for its half plus full K/V for the batch, then scores/softmax/PV
for its 1024 query rows.

All large matmuls run as fp8e4(e4m3) DoubleRow 3-term hi/lo splits:
    A @ B ~= Ah Bh + Al Bh + Ah Bl       (ll term dropped, ~0.07%/elem)
Each DoubleRow instruction contracts K=256 (two 128-chunks packed in the
free dim) at 0.5 cycles per output column, i.e. 4x the bf16 FLOP rate, so
the 3-term split costs 0.75x of bf16 with ~bf16 accuracy. Operands are kept
at std~1 so the lo plane stays clear of the fp8 subnormal floor: W ships as
32*W^T, x ships premultiplied by rstd (LN fold), and the q/k/v epilogue
rescales by 1/32 while splitting.

LayerNorm statistics, the W column sums, and the softmax max-shift are
folded on the host:
    qkv[d,t] = (G[d,t] + (-mu*rstd)[t]*wsum[d]) / 32,  G = (32 W'^T)(x rstd)
    position ships as (pos[i,j] - rowmax_j(pos[i,:]) - 1.5)/SCALE in bf16,
so exp(SCALE*psum) is range-safe for fp8 (max ~80 < 240) and the per-query
shift cancels between the PV numerator and the row-sum denominator.

Per-tile epilogues are spread across the non-PE engines (val on DVE, hi
cast on Act, lo subtract on Pool; exp on Act, position add on DVE) so the
tensor engine stream is the only critical path. Row sums accumulate in one
PSUM bank via 1-wide DoubleRow matmuls on the es hi/lo tiles.

Device layouts (per core):
    x_hi/x_lo   [C, N] f8      (x*rstd, token-rotated, hi/lo split)
    w_hi/w_lo   [C, 3C] f8     (32*W'^T, hi/lo split)
    nmr_b       [128, N] f8    (-mu*rstd, replicated rows)
    nmr_col     [128, NJT] f32 (-mu*rstd per token tile column)
    wsum        [128, 24] f32  (column sums of effective 32*W', [p, a])
    wvs_b       [128, C] f8    (v-part wsum, replicated rows)
    pos_t       [N, MY] bf16   ((pos - m_i)/SCALE, keys in local order)
    out         [MY, C] bf16   out[i, c] (host casts f32 + transposes)
"""

import os
import sys

for _p in ("/opt/trn_rl_repo",):
    if _p not in sys.path and os.path.isdir(_p):
        sys.path.insert(0, _p)

import numpy as np
import ml_dtypes

import concourse.bass as bass
import concourse.tile as tile
from concourse import bacc, mybir
from concourse.bass import ts, ds
from concourse.bass_utils import run_bass_kernel_spmd

FP = mybir.dt.float32
BF = mybir.dt.bfloat16
F8 = mybir.dt.float8e4
DR = mybir.MatmulPerfMode.DoubleRow
AF = mybir.ActivationFunctionType
MULT = mybir.AluOpType.mult
ADD = mybir.AluOpType.add
SUB = mybir.AluOpType.subtract

B = 4
C = 1024
N = 2048
MY = 1024  # query rows per core
D3 = 3 * C
NCH = C // 128   # 8 channel chunks
NCP = NCH // 2   # 4 channel chunk pairs
NJT = N // 128   # 16 key tiles
NJP = NJT // 2   # 8 key tile pairs
NIB = MY // 128  # 8 query blocks
LN_EPS = 1e-5
SCALE = 1.0 / np.sqrt(N)
WS = 32.0        # host weight pre-scale
M_SHIFT = -1.5   # softmax shift: m_i = rowmax + M_SHIFT, so P_max ~ e^(1.5+d)
                 # stays ~5..80 - high in fp8 range (floor noise) yet clip-safe


def build_kernel(rep=1, qk_bias=False, v_bias=False):
    nc = bacc.Bacc("TRN2", target_bir_lowering=False, debug=False, num_devices=8)
    xh_ext = nc.declare_dram_parameter("x_hi", [C, N], F8, isOutput=False)
    xl_ext = nc.declare_dram_parameter("x_lo", [C, N], F8, isOutput=False)
    wh_ext = nc.declare_dram_parameter("w_hi", [C, D3], F8, isOutput=False)
    wl_ext = nc.declare_dram_parameter("w_lo", [C, D3], F8, isOutput=False)
    nmrb_ext = nc.declare_dram_parameter("nmr_b", [128, N], F8, isOutput=False)
    nmrc_ext = nc.declare_dram_parameter("nmr_col", [128, NJT], FP, isOutput=False)
    ws_ext = nc.declare_dram_parameter("wsum", [128, 24], FP, isOutput=False)
    wvs_ext = nc.declare_dram_parameter("wvs_b", [128, C], F8, isOutput=False)
    b_ext = nc.declare_dram_parameter("bias", [128, 24], FP, isOutput=False)
    pos_ext = nc.declare_dram_parameter("pos_t", [N, MY], BF, isOutput=False)
    out_ext = nc.declare_dram_parameter("out", [MY, C], BF, isOutput=True)

    xh_r = xh_ext.ap().rearrange("(a p) n -> p a n", p=128)    # [128, 8, N]
    xl_r = xl_ext.ap().rearrange("(a p) n -> p a n", p=128)
    wh_r = wh_ext.ap().rearrange("(a p) d -> p a d", p=128)    # [128, 8, D3]
    wl_r = wl_ext.ap().rearrange("(a p) d -> p a d", p=128)

    with tile.TileContext(nc) as tc:
      for _r in range(rep):
        with (
            tc.tile_pool(name=f"res{_r}", bufs=1) as res,
            tc.tile_pool(name=f"statb{_r}", bufs=2) as statb,
            tc.tile_pool(name=f"pospool{_r}", bufs=3) as pospool,
            tc.tile_pool(name=f"valp{_r}", bufs=6) as valp,
            tc.tile_pool(name=f"rows{_r}", bufs=1) as rows,
            tc.tile_pool(name=f"psum{_r}", bufs=1, space="PSUM") as psum,
        ):
            # ---- resident tiles ----
            xh = res.tile([128, NCH, N], F8, tag="bigh", name="xh")
            xl = res.tile([128, NCH, N], F8, tag="bigl", name="xl")
            wqh = res.tile([128, NCH, 2 * C], F8, tag="wqh", name="wqh")
            wql = res.tile([128, NCH, 2 * C], F8, tag="wql", name="wql")
            wvh = res.tile([128, NCH, C], F8, tag="wvh", name="wvh")
            wvl = res.tile([128, NCH, C], F8, tag="wvl", name="wvl")
            qsh = res.tile([128, NCH, MY], F8, tag="qsh", name="qsh")
            qsl = res.tile([128, NCH, MY], F8, tag="qsl", name="qsl")
            ksh = res.tile([128, NCH, N], F8, tag="ksh", name="ksh")
            ksl = res.tile([128, NCH, N], F8, tag="ksl", name="ksl")
            vsh = res.tile([128, NJT, C], F8, tag="vsh", name="vsh")
            vsl = res.tile([128, NJT, C], F8, tag="vsl", name="vsl")

            ones2 = rows.tile([128, 2, 1], F8, tag="ones2", name="ones2")
            nc.vector.memset(ones2[:], 1.0)
            warm = rows.tile([128, 64], BF, tag="warm", name="warm")
            nc.vector.memset(warm[:], 0.0)

            nmr_b = rows.tile([128, N], F8, tag="nmrb", name="nmr_b")
            nmr_col = rows.tile([128, NJT], FP, tag="nmrc", name="nmr_col")
            wvsum_b = rows.tile([128, C], F8, tag="wvsb", name="wvsum_b")
            wsum_sb = rows.tile([128, 24], FP, tag="wsum", name="wsum_sb")
            bias_sb = rows.tile([128, 24], FP, tag="bias", name="bias_sb")

            # ---- input DMAs ----
            # One shared DMA device round-robins the three queues (SP / Act /
            # Pool), so spreading consumption-consecutive tensors across the
            # queues yields arrival in consumption order: xh0, wqh0, wql0,
            # xl0, wqh1, wql1, xh1, ... Stats ride the Act queue early enough
            # to unblock the first epilogues before PSUM pressure builds.
            nc.sync.dma_start(xh[:, :, ts(0, 512)], xh_r[:, :, ts(0, 512)])
            nc.sync.dma_start(xl[:, :, ts(0, 512)], xl_r[:, :, ts(0, 512)])
            nc.sync.dma_start(xh[:, :, ts(1, 512)], xh_r[:, :, ts(1, 512)])
            nc.sync.dma_start(xl[:, :, ts(1, 512)], xl_r[:, :, ts(1, 512)])
            nc.scalar.dma_start(wqh[:, :, ts(0, 512)], wh_r[:, :, ts(0, 512)])
            nc.scalar.dma_start(wqh[:, :, ts(1, 512)], wh_r[:, :, ts(1, 512)])
            nc.scalar.dma_start(wsum_sb[:], ws_ext.ap())
            nc.scalar.dma_start(bias_sb[:], b_ext.ap())
            nc.scalar.dma_start(nmr_col[:], nmrc_ext.ap())
            nc.gpsimd.dma_start(wql[:, :, ts(0, 512)],
                                wl_r[:, :, ts(0, 512)])
            nc.gpsimd.dma_start(nmr_b[:], nmrb_ext.ap())
            nc.gpsimd.dma_start(wql[:, :, ts(1, 512)],
                                wl_r[:, :, ts(1, 512)])

            # ---- PE ramp warm-up: burn the p-state window during DMA ----
            ps_w = psum.tile([128, 512], FP, tag="w", bufs=7, name="ps_w")
            ones_col = rows.tile([128, 1], BF, tag="onesc", name="ones_col")
            nc.vector.memset(ones_col[:], 0.0)
            for _ in range(120):
                nc.tensor.matmul(ps_w[0:1, ds(0, 64)], ones_col[:], warm[:],
                                 start=True, stop=True)

            # ---- 3-term DoubleRow contraction helper ----
            def mm3(ps, lh, ll, rh, rl, lslice, rslice, extra=0):
                """ps += (lh+ll).T (rh+rl) over all NCH chunks, 3 terms.
                lh/ll, rh/rl: [128, NCH, *] tiles; lslice/rslice: free slices.
                extra: count of further matmuls accumulating into ps after
                these (controls stop flag)."""
                k = 0
                for term in range(3):
                    lt = lh if term != 1 else ll
                    rt = rh if term != 2 else rl
                    for p in range(NCP):
                        nc.tensor.matmul(
                            ps, lt[:, ds(2 * p, 2), lslice],
                            rt[:, ds(2 * p, 2), rslice],
                            start=(k == 0), stop=(extra == 0 and k == 3 * NCP - 1),
                            perf_mode=DR)
                        k += 1

            # ---- q/k/v epilogue: val (DVE) -> hi (Act) -> lo (Pool) ----
            def qkv_epilogue(ps, dt, t, hi_dst, lo_dst, is_v=False, jt=None,
                             alt=False, defer=False):
                val = valp.tile([128, 512], BF, tag="val", name=f"val_{dt}_{t}")
                if defer:
                    # free the PSUM bank before nmr_b/wsum arrive: raw copy
                    # first, correction later from SBUF
                    vraw = valp.tile([128, 512], BF, tag="vraw",
                                     name=f"vraw_{dt}_{t}")
                    nc.vector.tensor_copy(vraw[:], ps)
                    nc.vector.scalar_tensor_tensor(
                        val[:], nmr_b[:, ts(t, 512)], wsum_sb[:, dt:dt + 1],
                        vraw[:], op0=MULT, op1=ADD)
                elif is_v:
                    nc.vector.scalar_tensor_tensor(
                        val[:], wvsum_b[:, ts(t, 512)], nmr_col[:, jt:jt + 1],
                        ps, op0=MULT, op1=ADD)
                else:
                    nc.vector.scalar_tensor_tensor(
                        val[:], nmr_b[:, ts(t, 512)], wsum_sb[:, dt:dt + 1],
                        ps, op0=MULT, op1=ADD)
                if (qk_bias and not is_v) or (v_bias and is_v):
                    # bias ships pre-scaled by 32 to match val's scale
                    if is_v:
                        # v bias varies along free dim; add via broadcast row
                        nc.vector.tensor_add(val[:], val[:],
                                             bias_v_b[:, ts(t, 512)])
                    else:
                        nc.vector.tensor_scalar_add(val[:], val[:],
                                                    bias_sb[:, dt:dt + 1])
                nc.scalar.mul(hi_dst, val[:], 1.0 / WS)
                if alt:
                    # Pool path: 2 ops, keeps the DVE under the PE tile rate
                    t32 = valp.tile([128, 512], BF, tag="t32",
                                    name=f"t32_{dt}_{t}")
                    nc.gpsimd.tensor_scalar_mul(t32[:], val[:], 1.0 / WS)
                    nc.gpsimd.tensor_sub(lo_dst, t32[:], hi_dst)
                else:
                    nc.vector.scalar_tensor_tensor(
                        lo_dst, val[:], 1.0 / WS, hi_dst, op0=MULT, op1=SUB)

            if v_bias:
                # bias_sb[:, 16:24] holds the v bias as [p, a] (d = a*128+p);
                # the v epilogue needs it along the free (c) dim, replicated
                # over token partitions: bounce through DRAM to transpose.
                bias_v_b = rows.tile([128, C], FP, tag="bvb", name="bias_v_b")
                bvd = nc.declare_dram_parameter("bias_vd", [1, C], FP,
                                                isOutput=True)
                nc.gpsimd.dma_start(
                    bvd.ap().rearrange("o (a p) -> (o p) a", p=128),
                    bias_sb[:, ds(16, 8)])
                bvrow = statb.tile([1, C], FP, tag="bvrow", bufs=1, name="bvrow")
                nc.gpsimd.dma_start(bvrow[:], bvd.ap())
                nc.gpsimd.partition_broadcast(bias_v_b[:], bvrow[:])

            # ---- Phase B1: q^T and k^T ----
            # q: dt 0..7 (d-slices of q), t 0..1 ; k: dt 8..15, t 0..3.
            # q-part first (w chunks 0-1), k-part after (chunks 2-3), each
            # t-outer, matching DMA arrival. Tiles run in groups of 4 with
            # term-sliced emission (all hh, then lh, then hl) so the wql/xl
            # DMAs get 1.7-3.4us of in-group slack.
            b1_tiles = ([(dt, t) for t in range(2) for dt in range(8)]
                        + [(dt, t) for t in range(4) for dt in range(8, 16)])
            for g in range(0, len(b1_tiles), 4):
                if g == 8:
                    # k-part weights + remaining x chunks: emitted here so
                    # their issue slots behind the q-part traffic
                    nc.scalar.dma_start(wqh[:, :, ts(2, 512)],
                                        wh_r[:, :, ts(2, 512)])
                    nc.gpsimd.dma_start(wql[:, :, ts(2, 512)],
                                        wl_r[:, :, ts(2, 512)])
                    nc.scalar.dma_start(wqh[:, :, ts(3, 512)],
                                        wh_r[:, :, ts(3, 512)])
                    nc.gpsimd.dma_start(wql[:, :, ts(3, 512)],
                                        wl_r[:, :, ts(3, 512)])
                    for t in range(2, 4):
                        nc.sync.dma_start(xh[:, :, ts(t, 512)],
                                          xh_r[:, :, ts(t, 512)])
                        nc.sync.dma_start(xl[:, :, ts(t, 512)],
                                          xl_r[:, :, ts(t, 512)])
                elif g == 24:
                    nc.scalar.dma_start(wvh[:], wh_r[:, :, ds(2 * C, C)])
                    nc.gpsimd.dma_start(wvl[:], wl_r[:, :, ds(2 * C, C)])
                    nc.scalar.dma_start(wvsum_b[:], wvs_ext.ap())
                group = b1_tiles[g:g + 4]
                pss = {}
                for dt, t in group:
                    pss[(dt, t)] = psum.tile([128, 512], FP, tag="w", bufs=7,
                                             name=f"qk_{dt}_{t}")
                for term in range(3):
                    lt = wqh if term != 1 else wql
                    rt = xh if term != 2 else xl
                    for dt, t in group:
                        for p in range(NCP):
                            nc.tensor.matmul(
                                pss[(dt, t)][:],
                                lt[:, ds(2 * p, 2), ds(dt * 128, 128)],
                                rt[:, ds(2 * p, 2), ts(t, 512)],
                                start=(term == 0 and p == 0),
                                stop=(term == 2 and p == NCP - 1),
                                perf_mode=DR)
                for gi, (dt, t) in enumerate(group):
                    if dt < 8:
                        hi = qsh[:, dt, ts(t, 512)]
                        lo = qsl[:, dt, ts(t, 512)]
                    else:
                        hi = ksh[:, dt - 8, ts(t, 512)]
                        lo = ksl[:, dt - 8, ts(t, 512)]
                    qkv_epilogue(pss[(dt, t)][:], dt, t, hi, lo,
                                 alt=(gi % 2 == 1))

            # ---- Phase B2: v (x stationary) ----
            for jt in range(NJT):
                for cc in range(2):
                    ps = psum.tile([128, 512], FP, tag="w", bufs=7,
                                   name=f"v_{jt}_{cc}")
                    mm3(ps[:], xh, xl, wvh, wvl, ts(jt, 128), ts(cc, 512))
                    qkv_epilogue(ps[:], 16 + jt, cc, vsh[:, jt, ts(cc, 512)],
                                 vsl[:, jt, ts(cc, 512)], is_v=True, jt=jt,
                                 alt=(cc == 1))

            # ---- Phase C: S^T = k^T.T q^T (+pos, exp) -> es hi/lo ----
            esh = res.tile([128, NJT, MY], F8, tag="bigh", name="esh")
            esl = res.tile([128, NJT, MY], F8, tag="bigl", name="esl")
            ps_sums = psum.tile([128, NIB], FP, tag="sums", bufs=1,
                                name="ps_sums")

            def rowsums(jp, first, last):
                # ps_sums[:, i] += sum over j-pair jp of es hi+lo rows
                for i in range(NIB):
                    nc.tensor.matmul(
                        ps_sums[:, i:i + 1], esh[:, ds(2 * jp, 2), ts(i, 128)],
                        ones2[:], start=(first and i == 0), stop=False,
                        perf_mode=DR)
                for i in range(NIB):
                    nc.tensor.matmul(
                        ps_sums[:, i:i + 1], esl[:, ds(2 * jp, 2), ts(i, 128)],
                        ones2[:], start=False, stop=(last and i == NIB - 1),
                        perf_mode=DR)

            for j in range(NJT):
                pos_tile = pospool.tile([128, MY], BF, tag="pos")
                nc.scalar.dma_start(pos_tile[:], pos_ext[ts(j, 128), :])
                pss = [psum.tile([128, 512], FP, tag="w", bufs=7,
                                 name=f"s_{j}_{ih}") for ih in range(2)]
                for ih in range(2):
                    mm3(pss[ih][:], ksh, ksl, qsh, qsl, ts(j, 128),
                        ts(ih, 512))
                if j >= 3 and j % 2 == 1:
                    # pair (j-3, j-2): two tiles of slack vs the Pool lo-sub
                    rowsums((j - 3) // 2, first=(j == 3), last=False)
                esvs = []
                for ih in range(2):
                    nc.vector.tensor_add(pss[ih][:], pss[ih][:],
                                         pos_tile[:, ts(ih, 512)])
                for ih in range(2):
                    esv = valp.tile([128, 512], BF, tag="esv",
                                    name=f"esv_{j}_{ih}")
                    nc.scalar.activation(esv[:], pss[ih][:], AF.Exp,
                                         scale=SCALE)
                    esvs.append(esv)
                for ih in range(2):
                    if ih == 0 or j == NJT - 1:
                        nc.scalar.copy(esh[:, j, ts(ih, 512)], esvs[ih][:])
                    else:
                        nc.vector.tensor_copy(esh[:, j, ts(ih, 512)],
                                              esvs[ih][:])
                    if j == NJT - 1:
                        # last tile: phase D waits on these; DVE is faster
                        nc.vector.tensor_sub(esl[:, j, ts(ih, 512)],
                                             esvs[ih][:],
                                             esh[:, j, ts(ih, 512)])
                    else:
                        nc.gpsimd.tensor_sub(esl[:, j, ts(ih, 512)],
                                             esvs[ih][:],
                                             esh[:, j, ts(ih, 512)])

            # ---- Phase D: out[i, c] = (P^T)^T v / rowsum ----
            recips = rows.tile([128, NIB], FP, tag="recips", name="recips")

            def pv(ps, i, off, width=512, tail_cb=None):
                # pairs 0..6 of every term first; the (14, 15) pair last so
                # the PE has ~2us of work before needing the final es tiles
                seq = ([(term, p) for term in range(3) for p in range(NJP - 1)]
                       + [(term, NJP - 1) for term in range(3)])
                for k, (term, p) in enumerate(seq):
                    if k == 3 * (NJP - 1) and tail_cb is not None:
                        tail_cb()
                    et = esh if term != 1 else esl
                    vt = vsh if term != 2 else vsl
                    nc.tensor.matmul(
                        ps, et[:, ds(2 * p, 2), ts(i, 128)],
                        vt[:, ds(2 * p, 2), ds(off, width)],
                        start=(k == 0), stop=(k == len(seq) - 1),
                        perf_mode=DR)

            for i in range(NIB):
                pso = [psum.tile([128, 512], FP, tag="w", bufs=7,
                                 name=f"o_{i}_{cc}") for cc in range(2)]
                if i == 0:
                    # last rowsum pair (14, 15) slots in after the pair-0..6
                    # PV matmuls; reciprocals follow
                    pv(pso[0][:], i, 0,
                       tail_cb=lambda: rowsums(NJP - 1, first=False, last=True))
                    nc.vector.reciprocal(recips[:], ps_sums[:])
                else:
                    pv(pso[0][:], i, 0)
                out_t = statb.tile([128, C], BF, tag="statb", bufs=2,
                                   name=f"out_t{i}")
                if i < NIB - 1:
                    pv(pso[1][:], i, 512)
                    nc.scalar.mul(out_t[:, ts(0, 512)], pso[0][:],
                                  recips[:, i:i + 1])
                    nc.sync.dma_start(out_ext[ts(i, 128), ts(0, 512)],
                                      out_t[:, ts(0, 512)])
                    nc.scalar.mul(out_t[:, ts(1, 512)], pso[1][:],
                                  recips[:, i:i + 1])
                    nc.sync.dma_start(out_ext[ts(i, 128), ts(1, 512)],
                                      out_t[:, ts(1, 512)])
                else:
                    # final block: 384-wide slice drains while the last
                    # 128-wide slice computes, shortening the kernel tail
                    nc.scalar.mul(out_t[:, ts(0, 512)], pso[0][:],
                                  recips[:, i:i + 1])
                    nc.sync.dma_start(out_ext[ts(i, 128), ts(0, 512)],
                                      out_t[:, ts(0, 512)])
                    pv(pso[1][:, ds(0, 384)], i, 512, width=384)
                    nc.scalar.mul(out_t[:, ds(512, 384)],
                                  pso[1][:, ds(0, 384)], recips[:, i:i + 1])
                    nc.sync.dma_start(out_ext[ts(i, 128), ds(512, 384)],
                                      out_t[:, ds(512, 384)])
                    ps_f = psum.tile([128, 512], FP, tag="w", bufs=7,
                                     name="ps_fin")
                    pv(ps_f[:, ds(0, 128)], i, 896, width=128)
                    nc.scalar.mul(out_t[:, ds(896, 128)], ps_f[:, ds(0, 128)],
                                  recips[:, i:i + 1])
                    nc.sync.dma_start(out_ext[ts(i, 128), ds(896, 128)],
                                      out_t[:, ds(896, 128)])

    nc.compile()
    return nc


_NC_CACHE = {}


def _get_nc(qk_bias, v_bias):
    key = (qk_bias, v_bias)
    if key not in _NC_CACHE:
        _NC_CACHE[key] = build_kernel(qk_bias=qk_bias, v_bias=v_bias)
    return _NC_CACHE[key]


def _split8(a):
    hi32 = np.clip(a, -240, 240).astype(ml_dtypes.float8_e4m3)
    lo = (a - hi32.astype(np.float32)).astype(ml_dtypes.float8_e4m3)
    return hi32, lo


def prep_in_maps(x, position, ln_gamma, ln_beta, W_qkv, b_qkv):
    """Host-side sharding / layout prep. Returns in_maps for 8 cores."""
    x = np.asarray(x, dtype=np.float32)
    position = np.asarray(position, dtype=np.float32)
    ln_gamma = np.asarray(ln_gamma, dtype=np.float32)
    ln_beta = np.asarray(ln_beta, dtype=np.float32)
    W_qkv = np.asarray(W_qkv, dtype=np.float32)
    b_qkv = np.asarray(b_qkv, dtype=np.float32)

    # Fold gamma into W columns, beta into bias. SCALE is applied at exp.
    # bias ships pre-scaled by WS to match the 32x val scale in the epilogue.
    Wp = W_qkv * ln_gamma[None, :]
    bp = (WS * (b_qkv + W_qkv @ ln_beta)).copy()
    Ws = np.ascontiguousarray(WS * Wp.T)          # [C, 3C]
    w_hi, w_lo = _split8(Ws)
    weff = w_hi.astype(np.float32) + w_lo.astype(np.float32)
    wsum = np.ascontiguousarray(weff.sum(axis=0), dtype=np.float32)
    wsum_2d = np.ascontiguousarray(wsum.reshape(24, 128).T)
    wvs_b = np.broadcast_to(np.clip(wsum[2 * C:], -240, 240).astype(
        ml_dtypes.float8_e4m3), (128, C)).copy()

    # position: per-query max-shift + 1/SCALE scaling, bf16
    m = position.max(axis=1) + M_SHIFT            # [N] per query i
    posp = (position - m[:, None]) / SCALE        # [i, j]

    in_maps = []
    for core in range(8):
        b, s = divmod(core, 2)
        xb = x[b]
        mu = xb.mean(axis=0)
        var = ((xb - mu) ** 2).mean(axis=0)
        rstd = 1.0 / np.sqrt(var + LN_EPS)
        if s == 1:
            xb = np.roll(xb, -MY, axis=1)
            mu = np.roll(mu, -MY)
            rstd = np.roll(rstd, -MY)
            pos_rot = np.roll(posp, -MY, axis=1)
        else:
            pos_rot = posp
        xr = xb * rstd[None, :]
        x_hi, x_lo = _split8(xr)
        nmr = np.clip(-mu * rstd, -240, 240).astype(ml_dtypes.float8_e4m3)
        nmr_b = np.broadcast_to(nmr, (128, N)).copy()
        nmr_col = np.ascontiguousarray(
            (-mu * rstd).reshape(NJT, 128).T, dtype=np.float32)
        pos_t = np.ascontiguousarray(
            pos_rot[s * MY:(s + 1) * MY, :].T).astype(ml_dtypes.bfloat16)
        in_maps.append({
            "x_hi": x_hi, "x_lo": x_lo,
            "w_hi": w_hi, "w_lo": w_lo,
            "nmr_b": nmr_b, "nmr_col": nmr_col,
            "wsum": wsum_2d, "wvs_b": wvs_b,
            "bias": np.ascontiguousarray(bp.reshape(24, 128).T),
            "pos_t": pos_t,
        })
    return in_maps


def kernel(x, position, ln_gamma, ln_beta, W_qkv, b_qkv):
    in_maps = prep_in_maps(x, position, ln_gamma, ln_beta, W_qkv, b_qkv)
    bp = in_maps[0]["bias"]  # [128, 24]: cols 0:16 are q,k; 16:24 are v
    nc = _get_nc(bool(np.abs(bp[:, :16]).max() > 0),
                 bool(np.abs(bp[:, 16:]).max() > 0))
    res = run_bass_kernel_spmd(nc, in_maps, core_ids=list(range(8)))
    out = np.empty((B, C, N), dtype=np.float32)
    for core in range(8):
        b, s = divmod(core, 2)
        out[b, :, s * MY:(s + 1) * MY] = res.results[core]["out"].astype(np.float32).T
    return out


# revision 40
# speedup vs baseline: 1.0150x; 1.0031x over previous
"""Trainium2 Bass kernel for nn_Attention_54030688584207.

Single-head attention block:
    h = LN(x^T) ; qkv = h @ W^T + b ; S = q k^T / sqrt(N) + position
    out = softmax(S) @ v, returned as [B, C, N].

Sharding: 8 cores = 4 batches x 2 query-halves, no collectives. Each core
receives its batch's x rotated so its own 1024 query tokens come first and
computes q for its half plus full K/V for the batch, then scores/softmax/PV
for its 1024 query rows.

All large matmuls run as fp8e4(e4m3) DoubleRow 3-term hi/lo splits:
    A @ B ~= Ah Bh + Al Bh + Ah Bl       (ll term dropped, ~0.07%/elem)
Each DoubleRow instruction contracts K=256 (two 128-chunks packed in the
free dim) at 0.5 cycles per output column, i.e. 4x the bf16 FLOP rate, so
the 3-term split costs 0.75x of bf16 with ~bf16 accuracy. Operands are kept
at std~1 so the lo plane stays clear of the fp8 subnormal floor: W ships as
32*W^T, x ships premultiplied by rstd (LN fold), and the q/k/v epilogue
rescales by 1/32 while splitting.

LayerNorm statistics, the W column sums, and the softmax max-shift are
folded on the host:
    qkv[d,t] = (G[d,t] + (-mu*rstd)[t]*wsum[d]) / 32,  G = (32 W'^T)(x rstd)
    position ships as (pos[i,j] - rowmax_j(pos[i,:]) - 1.5)/SCALE in bf16,
so exp(SCALE*psum) is range-safe for fp8 (max ~80 < 240) and the per-query
shift cancels between the PV numerator and the row-sum denominator.

Per-tile epilogues are spread across the non-PE engines (val on DVE, hi
cast on Act, lo subtract on Pool; exp on Act, position add on DVE) so the
tensor engine stream is the only critical path. Row sums accumulate in one
PSUM bank via 1-wide DoubleRow matmuls on the es hi/lo tiles.

Device layouts (per core):
    x_hi/x_lo   [C, N] f8      (x*rstd, token-rotated, hi/lo split)
    w_hi/w_lo   [C, 3C] f8     (32*W'^T, hi/lo split)
    nmr_b       [128, N] f8    (-mu*rstd, replicated rows)
    nmr_col     [128, NJT] f32 (-mu*rstd per token tile column)
    wsum        [128, 24] f32  (column sums of effective 32*W', [p, a])
    wvs_b       [128, C] f8    (v-part wsum, replicated rows)
    pos_t       [N, MY] bf16   ((pos - m_i)/SCALE, keys in local order)
    out         [MY, C] bf16   out[i, c] (host casts f32 + transposes)
"""

import os
import sys

for _p in ("/opt/trn_rl_repo",):
    if _p not in sys.path and os.path.isdir(_p):
        sys.path.insert(0, _p)

import numpy as np
import ml_dtypes

import concourse.bass as bass
import concourse.tile as tile
from concourse import bacc, mybir
from concourse.bass import ts, ds
from concourse.bass_utils import run_bass_kernel_spmd

FP = mybir.dt.float32
BF = mybir.dt.bfloat16
F8 = mybir.dt.float8e4
DR = mybir.MatmulPerfMode.DoubleRow
AF = mybir.ActivationFunctionType
MULT = mybir.AluOpType.mult
ADD = mybir.AluOpType.add
SUB = mybir.AluOpType.subtract

B = 4
C = 1024
N = 2048
MY = 1024  # query rows per core
D3 = 3 * C
NCH = C // 128   # 8 channel chunks
NCP = NCH // 2   # 4 channel chunk pairs
NJT = N // 128   # 16 key tiles
NJP = NJT // 2   # 8 key tile pairs
NIB = MY // 128  # 8 query blocks
LN_EPS = 1e-5
SCALE = 1.0 / np.sqrt(N)
WS = 32.0        # host weight pre-scale
M_SHIFT = -1.5   # softmax shift: m_i = rowmax + M_SHIFT, so P_max ~ e^(1.5+d)
                 # stays ~5..80 - high in fp8 range (floor noise) yet clip-safe


def build_kernel(rep=1, qk_bias=False, v_bias=False):
    nc = bacc.Bacc("TRN2", target_bir_lowering=False, debug=False, num_devices=8)
    xh_ext = nc.declare_dram_parameter("x_hi", [C, N], F8, isOutput=False)
    xl_ext = nc.declare_dram_parameter("x_lo", [C, N], F8, isOutput=False)
    wh_ext = nc.declare_dram_parameter("w_hi", [C, D3], F8, isOutput=False)
    wl_ext = nc.declare_dram_parameter("w_lo", [C, D3], F8, isOutput=False)
    nmrb_ext = nc.declare_dram_parameter("nmr_b", [128, N], F8, isOutput=False)
    nmrc_ext = nc.declare_dram_parameter("nmr_col", [128, NJT], FP, isOutput=False)
    ws_ext = nc.declare_dram_parameter("wsum", [128, 24], FP, isOutput=False)
    wvs_ext = nc.declare_dram_parameter("wvs_b", [128, C], F8, isOutput=False)
    b_ext = nc.declare_dram_parameter("bias", [128, 24], FP, isOutput=False)
    pos_ext = nc.declare_dram_parameter("pos_t", [N, MY], BF, isOutput=False)
    out_ext = nc.declare_dram_parameter("out", [MY, C], BF, isOutput=True)

    xh_r = xh_ext.ap().rearrange("(a p) n -> p a n", p=128)    # [128, 8, N]
    xl_r = xl_ext.ap().rearrange("(a p) n -> p a n", p=128)
    wh_r = wh_ext.ap().rearrange("(a p) d -> p a d", p=128)    # [128, 8, D3]
    wl_r = wl_ext.ap().rearrange("(a p) d -> p a d", p=128)

    with tile.TileContext(nc) as tc:
      for _r in range(rep):
        with (
            tc.tile_pool(name=f"res{_r}", bufs=1) as res,
            tc.tile_pool(name=f"statb{_r}", bufs=2) as statb,
            tc.tile_pool(name=f"pospool{_r}", bufs=3) as pospool,
            tc.tile_pool(name=f"valp{_r}", bufs=6) as valp,
            tc.tile_pool(name=f"rows{_r}", bufs=1) as rows,
            tc.tile_pool(name=f"psum{_r}", bufs=1, space="PSUM") as psum,
        ):
            # ---- resident tiles ----
            xh = res.tile([128, NCH, N], F8, tag="bigh", name="xh")
            xl = res.tile([128, NCH, N], F8, tag="bigl", name="xl")
            wqh = res.tile([128, NCH, 2 * C], F8, tag="wqh", name="wqh")
            wql = res.tile([128, NCH, 2 * C], F8, tag="wql", name="wql")
            wvh = res.tile([128, NCH, C], F8, tag="wvh", name="wvh")
            wvl = res.tile([128, NCH, C], F8, tag="wvl", name="wvl")
            qsh = res.tile([128, NCH, MY], F8, tag="qsh", name="qsh")
            qsl = res.tile([128, NCH, MY], F8, tag="qsl", name="qsl")
            ksh = res.tile([128, NCH, N], F8, tag="ksh", name="ksh")
            ksl = res.tile([128, NCH, N], F8, tag="ksl", name="ksl")
            vsh = res.tile([128, NJT, C], F8, tag="vsh", name="vsh")
            vsl = res.tile([128, NJT, C], F8, tag="vsl", name="vsl")

            ones2 = rows.tile([128, 2, 1], F8, tag="ones2", name="ones2")
            nc.vector.memset(ones2[:], 1.0)
            warm = rows.tile([128, 64], BF, tag="warm", name="warm")
            nc.vector.memset(warm[:], 0.0)

            nmr_b = rows.tile([128, N], F8, tag="nmrb", name="nmr_b")
            nmr_col = rows.tile([128, NJT], FP, tag="nmrc", name="nmr_col")
            wvsum_b = rows.tile([128, C], F8, tag="wvsb", name="wvsum_b")
            wsum_sb = rows.tile([128, 24], FP, tag="wsum", name="wsum_sb")
            bias_sb = rows.tile([128, 24], FP, tag="bias", name="bias_sb")

            # ---- input DMAs ----
            # One shared DMA device round-robins the three queues (SP / Act /
            # Pool), so spreading consumption-consecutive tensors across the
            # queues yields arrival in consumption order: xh0, wqh0, wql0,
            # xl0, wqh1, wql1, xh1, ... Stats ride the Act queue early enough
            # to unblock the first epilogues before PSUM pressure builds.
            nc.sync.dma_start(xh[:, :, ts(0, 512)], xh_r[:, :, ts(0, 512)])
            nc.sync.dma_start(xl[:, :, ts(0, 512)], xl_r[:, :, ts(0, 512)])
            nc.sync.dma_start(xh[:, :, ts(1, 512)], xh_r[:, :, ts(1, 512)])
            nc.sync.dma_start(xl[:, :, ts(1, 512)], xl_r[:, :, ts(1, 512)])
            nc.scalar.dma_start(wqh[:, :, ts(0, 512)], wh_r[:, :, ts(0, 512)])
            nc.scalar.dma_start(wqh[:, :, ts(1, 512)], wh_r[:, :, ts(1, 512)])
            nc.scalar.dma_start(wsum_sb[:], ws_ext.ap())
            nc.scalar.dma_start(bias_sb[:], b_ext.ap())
            nc.scalar.dma_start(nmr_col[:], nmrc_ext.ap())
            nc.gpsimd.dma_start(wql[:, :, ts(0, 512)],
                                wl_r[:, :, ts(0, 512)])
            nc.gpsimd.dma_start(nmr_b[:], nmrb_ext.ap())
            nc.gpsimd.dma_start(wql[:, :, ts(1, 512)],
                                wl_r[:, :, ts(1, 512)])

            # ---- PE ramp warm-up: burn the p-state window during DMA ----
            ps_w = psum.tile([128, 512], FP, tag="w", bufs=7, name="ps_w")
            ones_col = rows.tile([128, 1], BF, tag="onesc", name="ones_col")
            nc.vector.memset(ones_col[:], 0.0)
            for _ in range(120):
                nc.tensor.matmul(ps_w[0:1, ds(0, 64)], ones_col[:], warm[:],
                                 start=True, stop=True)

            # ---- 3-term DoubleRow contraction helper ----
            def mm3(ps, lh, ll, rh, rl, lslice, rslice, extra=0):
                """ps += (lh+ll).T (rh+rl) over all NCH chunks, 3 terms.
                lh/ll, rh/rl: [128, NCH, *] tiles; lslice/rslice: free slices.
                extra: count of further matmuls accumulating into ps after
                these (controls stop flag)."""
                k = 0
                for term in range(3):
                    lt = lh if term != 1 else ll
                    rt = rh if term != 2 else rl
                    for p in range(NCP):
                        nc.tensor.matmul(
                            ps, lt[:, ds(2 * p, 2), lslice],
                            rt[:, ds(2 * p, 2), rslice],
                            start=(k == 0), stop=(extra == 0 and k == 3 * NCP - 1),
                            perf_mode=DR)
                        k += 1

            # ---- q/k/v epilogue: val (DVE) -> hi (Act) -> lo (Pool) ----
            def qkv_epilogue(ps, dt, t, hi_dst, lo_dst, is_v=False, jt=None,
                             alt=False, defer=False):
                val = valp.tile([128, 512], BF, tag="val", name=f"val_{dt}_{t}")
                if defer:
                    # free the PSUM bank before nmr_b/wsum arrive: raw copy
                    # first, correction later from SBUF
                    vraw = valp.tile([128, 512], BF, tag="vraw",
                                     name=f"vraw_{dt}_{t}")
                    nc.vector.tensor_copy(vraw[:], ps)
                    nc.vector.scalar_tensor_tensor(
                        val[:], nmr_b[:, ts(t, 512)], wsum_sb[:, dt:dt + 1],
                        vraw[:], op0=MULT, op1=ADD)
                elif is_v:
                    nc.vector.scalar_tensor_tensor(
                        val[:], wvsum_b[:, ts(t, 512)], nmr_col[:, jt:jt + 1],
                        ps, op0=MULT, op1=ADD)
                else:
                    nc.vector.scalar_tensor_tensor(
                        val[:], nmr_b[:, ts(t, 512)], wsum_sb[:, dt:dt + 1],
                        ps, op0=MULT, op1=ADD)
                if (qk_bias and not is_v) or (v_bias and is_v):
                    # bias ships pre-scaled by 32 to match val's scale
                    if is_v:
                        # v bias varies along free dim; add via broadcast row
                        nc.vector.tensor_add(val[:], val[:],
                                             bias_v_b[:, ts(t, 512)])
                    else:
                        nc.vector.tensor_scalar_add(val[:], val[:],
                                                    bias_sb[:, dt:dt + 1])
                nc.scalar.mul(hi_dst, val[:], 1.0 / WS)
                if alt:
                    # Pool path: 2 ops, keeps the DVE under the PE tile rate
                    t32 = valp.tile([128, 512], BF, tag="t32",
                                    name=f"t32_{dt}_{t}")
                    nc.gpsimd.tensor_scalar_mul(t32[:], val[:], 1.0 / WS)
                    nc.gpsimd.tensor_sub(lo_dst, t32[:], hi_dst)
                else:
                    nc.vector.scalar_tensor_tensor(
                        lo_dst, val[:], 1.0 / WS, hi_dst, op0=MULT, op1=SUB)

            if v_bias:
                # bias_sb[:, 16:24] holds the v bias as [p, a] (d = a*128+p);
                # the v epilogue needs it along the free (c) dim, replicated
                # over token partitions: bounce through DRAM to transpose.
                bias_v_b = rows.tile([128, C], FP, tag="bvb", name="bias_v_b")
                bvd = nc.declare_dram_parameter("bias_vd", [1, C], FP,
                                                isOutput=True)
                nc.gpsimd.dma_start(
                    bvd.ap().rearrange("o (a p) -> (o p) a", p=128),
                    bias_sb[:, ds(16, 8)])
                bvrow = statb.tile([1, C], FP, tag="bvrow", bufs=1, name="bvrow")
                nc.gpsimd.dma_start(bvrow[:], bvd.ap())
                nc.gpsimd.partition_broadcast(bias_v_b[:], bvrow[:])

            # ---- Phase B1: q^T and k^T ----
            # q: dt 0..7 (d-slices of q), t 0..1 ; k: dt 8..15, t 0..3.
            # q-part first (w chunks 0-1), k-part after (chunks 2-3), each
            # t-outer, matching DMA arrival. Tiles run in groups of 4 with
            # term-sliced emission (all hh, then lh, then hl) so the wql/xl
            # DMAs get 1.7-3.4us of in-group slack.
            b1_tiles = ([(dt, t) for t in range(2) for dt in range(8)]
                        + [(dt, t) for t in range(4) for dt in range(8, 16)])
            for g in range(0, len(b1_tiles), 4):
                if g == 8:
                    # k-part weights + remaining x chunks: emitted here so
                    # their issue slots behind the q-part traffic
                    nc.scalar.dma_start(wqh[:, :, ts(2, 512)],
                                        wh_r[:, :, ts(2, 512)])
                    nc.gpsimd.dma_start(wql[:, :, ts(2, 512)],
                                        wl_r[:, :, ts(2, 512)])
                    nc.scalar.dma_start(wqh[:, :, ts(3, 512)],
                                        wh_r[:, :, ts(3, 512)])
                    nc.gpsimd.dma_start(wql[:, :, ts(3, 512)],
                                        wl_r[:, :, ts(3, 512)])
                    for t in range(2, 4):
                        nc.sync.dma_start(xh[:, :, ts(t, 512)],
                                          xh_r[:, :, ts(t, 512)])
                        nc.sync.dma_start(xl[:, :, ts(t, 512)],
                                          xl_r[:, :, ts(t, 512)])
                elif g == 24:
                    nc.scalar.dma_start(wvh[:], wh_r[:, :, ds(2 * C, C)])
                    nc.gpsimd.dma_start(wvl[:], wl_r[:, :, ds(2 * C, C)])
                    nc.scalar.dma_start(wvsum_b[:], wvs_ext.ap())
                group = b1_tiles[g:g + 4]
                pss = {}
                for dt, t in group:
                    pss[(dt, t)] = psum.tile([128, 512], FP, tag="w", bufs=7,
                                             name=f"qk_{dt}_{t}")
                for term in range(3):
                    lt = wqh if term != 1 else wql
                    rt = xh if term != 2 else xl
                    for dt, t in group:
                        for p in range(NCP):
                            nc.tensor.matmul(
                                pss[(dt, t)][:],
                                lt[:, ds(2 * p, 2), ds(dt * 128, 128)],
                                rt[:, ds(2 * p, 2), ts(t, 512)],
                                start=(term == 0 and p == 0),
                                stop=(term == 2 and p == NCP - 1),
                                perf_mode=DR)
                for gi, (dt, t) in enumerate(group):
                    if dt < 8:
                        hi = qsh[:, dt, ts(t, 512)]
                        lo = qsl[:, dt, ts(t, 512)]
                    else:
                        hi = ksh[:, dt - 8, ts(t, 512)]
                        lo = ksl[:, dt - 8, ts(t, 512)]
                    qkv_epilogue(pss[(dt, t)][:], dt, t, hi, lo,
                                 alt=(gi % 2 == 1))

            # ---- Phase B2: v (x stationary) ----
            for jt in range(NJT):
                for cc in range(2):
                    ps = psum.tile([128, 512], FP, tag="w", bufs=7,
                                   name=f"v_{jt}_{cc}")
                    mm3(ps[:], xh, xl, wvh, wvl, ts(jt, 128), ts(cc, 512))
                    qkv_epilogue(ps[:], 16 + jt, cc, vsh[:, jt, ts(cc, 512)],
                                 vsl[:, jt, ts(cc, 512)], is_v=True, jt=jt,
                                 alt=(cc == 1))

            # ---- Phase C: S^T = k^T.T q^T (+pos, exp) -> es hi/lo ----
            esh = res.tile([128, NJT, MY], F8, tag="bigh", name="esh")
            esl = res.tile([128, NJT, MY], F8, tag="bigl", name="esl")
            ps_sums = psum.tile([128, NIB], FP, tag="sums", bufs=1,
                                name="ps_sums")

            def rowsums(jp, first, last):
                # ps_sums[:, i] += sum over j-pair jp of es hi+lo rows
                for i in range(NIB):
                    nc.tensor.matmul(
                        ps_sums[:, i:i + 1], esh[:, ds(2 * jp, 2), ts(i, 128)],
                        ones2[:], start=(first and i == 0), stop=False,
                        perf_mode=DR)
                for i in range(NIB):
                    nc.tensor.matmul(
                        ps_sums[:, i:i + 1], esl[:, ds(2 * jp, 2), ts(i, 128)],
                        ones2[:], start=False, stop=(last and i == NIB - 1),
                        perf_mode=DR)

            for j in range(NJT):
                pos_tile = pospool.tile([128, MY], BF, tag="pos")
                nc.scalar.dma_start(pos_tile[:], pos_ext[ts(j, 128), :])
                pss = [psum.tile([128, 512], FP, tag="w", bufs=7,
                                 name=f"s_{j}_{ih}") for ih in range(2)]
                for ih in range(2):
                    mm3(pss[ih][:], ksh, ksl, qsh, qsl, ts(j, 128),
                        ts(ih, 512))
                if j >= 3 and j % 2 == 1:
                    # pair (j-3, j-2): two tiles of slack vs the Pool lo-sub
                    rowsums((j - 3) // 2, first=(j == 3), last=False)
                esvs = []
                for ih in range(2):
                    nc.vector.tensor_add(pss[ih][:], pss[ih][:],
                                         pos_tile[:, ts(ih, 512)])
                for ih in range(2):
                    esv = valp.tile([128, 512], BF, tag="esv",
                                    name=f"esv_{j}_{ih}")
                    nc.scalar.activation(esv[:], pss[ih][:], AF.Exp,
                                         scale=SCALE)
                    esvs.append(esv)
                for ih in range(2):
                    if ih == 0 or j == NJT - 1:
                        nc.scalar.copy(esh[:, j, ts(ih, 512)], esvs[ih][:])
                    else:
                        nc.vector.tensor_copy(esh[:, j, ts(ih, 512)],
                                              esvs[ih][:])
                    if j == NJT - 1:
                        # last tile: phase D waits on these; DVE is faster
                        nc.vector.tensor_sub(esl[:, j, ts(ih, 512)],
                                             esvs[ih][:],
                                             esh[:, j, ts(ih, 512)])
                    else:
                        nc.gpsimd.tensor_sub(esl[:, j, ts(ih, 512)],
                                             esvs[ih][:],
                                             esh[:, j, ts(ih, 512)])

            # ---- Phase D: out[i, c] = (P^T)^T v / rowsum ----
            recips = rows.tile([128, NIB], FP, tag="recips", name="recips")

            def pv(ps, i, off, width=512, tail_cb=None):
                # pairs 0..6 of every term first; the (14, 15) pair last so
                # the PE has ~2us of work before needing the final es tiles
                seq = ([(term, p) for term in range(3) for p in range(NJP - 1)]
                       + [(term, NJP - 1) for term in range(3)])
                for k, (term, p) in enumerate(seq):
                    if k == 3 * (NJP - 1) and tail_cb is not None:
                        tail_cb()
                    et = esh if term != 1 else esl
                    vt = vsh if term != 2 else vsl
                    nc.tensor.matmul(
                        ps, et[:, ds(2 * p, 2), ts(i, 128)],
                        vt[:, ds(2 * p, 2), ds(off, width)],
                        start=(k == 0), stop=(k == len(seq) - 1),
                        perf_mode=DR)

            # i=0/1 cc=0 interleaved: both tiles' pair-0..6 matmuls run
            # before anything needs the final es tiles, bridging the tail of
            # the last exp/split chain with ~4.5us of PE work
            ps01 = [psum.tile([128, 512], FP, tag="w", bufs=7,
                              name=f"o01_{i}") for i in range(2)]
            pv(ps01[0][:], 0, 0,
               tail_cb=lambda: (pv(ps01[1][:], 1, 0,
                                   tail_cb=lambda: rowsums(NJP - 1,
                                                           first=False,
                                                           last=True)),))
            nc.vector.reciprocal(recips[:], ps_sums[:])
            for i in range(NIB):
                pso = [ps01[i] if cc == 0 and i < 2 else
                       psum.tile([128, 512], FP, tag="w", bufs=7,
                                 name=f"o_{i}_{cc}") for cc in range(2)]
                if i >= 2:
                    pv(pso[0][:], i, 0)
                out_t = statb.tile([128, C], BF, tag="statb", bufs=2,
                                   name=f"out_t{i}")
                if i < NIB - 1:
                    pv(pso[1][:], i, 512)
                    nc.scalar.mul(out_t[:, ts(0, 512)], pso[0][:],
                                  recips[:, i:i + 1])
                    nc.sync.dma_start(out_ext[ts(i, 128), ts(0, 512)],
                                      out_t[:, ts(0, 512)])
                    nc.scalar.mul(out_t[:, ts(1, 512)], pso[1][:],
                                  recips[:, i:i + 1])
                    nc.sync.dma_start(out_ext[ts(i, 128), ts(1, 512)],
                                      out_t[:, ts(1, 512)])
                else:
                    # final block: 384-wide slice drains while the last
                    # 128-wide slice computes, shortening the kernel tail
                    nc.scalar.mul(out_t[:, ts(0, 512)], pso[0][:],
                                  recips[:, i:i + 1])
                    nc.sync.dma_start(out_ext[ts(i, 128), ts(0, 512)],
                                      out_t[:, ts(0, 512)])
                    pv(pso[1][:, ds(0, 384)], i, 512, width=384)
                    nc.scalar.mul(out_t[:, ds(512, 384)],
                                  pso[1][:, ds(0, 384)], recips[:, i:i + 1])
                    nc.sync.dma_start(out_ext[ts(i, 128), ds(512, 384)],
                                      out_t[:, ds(512, 384)])
                    ps_f = psum.tile([128, 512], FP, tag="w", bufs=7,
                                     name="ps_fin")
                    pv(ps_f[:, ds(0, 128)], i, 896, width=128)
                    nc.scalar.mul(out_t[:, ds(896, 128)], ps_f[:, ds(0, 128)],
                                  recips[:, i:i + 1])
                    nc.sync.dma_start(out_ext[ts(i, 128), ds(896, 128)],
                                      out_t[:, ds(896, 128)])

    nc.compile()
    return nc


_NC_CACHE = {}


def _get_nc(qk_bias, v_bias):
    key = (qk_bias, v_bias)
    if key not in _NC_CACHE:
        _NC_CACHE[key] = build_kernel(qk_bias=qk_bias, v_bias=v_bias)
    return _NC_CACHE[key]


def _split8(a):
    hi32 = np.clip(a, -240, 240).astype(ml_dtypes.float8_e4m3)
    lo = (a - hi32.astype(np.float32)).astype(ml_dtypes.float8_e4m3)
    return hi32, lo


def prep_in_maps(x, position, ln_gamma, ln_beta, W_qkv, b_qkv):
    """Host-side sharding / layout prep. Returns in_maps for 8 cores."""
    x = np.asarray(x, dtype=np.float32)
    position = np.asarray(position, dtype=np.float32)
    ln_gamma = np.asarray(ln_gamma, dtype=np.float32)
    ln_beta = np.asarray(ln_beta, dtype=np.float32)
    W_qkv = np.asarray(W_qkv, dtype=np.float32)
    b_qkv = np.asarray(b_qkv, dtype=np.float32)

    # Fold gamma into W columns, beta into bias. SCALE is applied at exp.
    # bias ships pre-scaled by WS to match the 32x val scale in the epilogue.
    Wp = W_qkv * ln_gamma[None, :]
    bp = (WS * (b_qkv + W_qkv @ ln_beta)).copy()
    Ws = np.ascontiguousarray(WS * Wp.T)          # [C, 3C]
    w_hi, w_lo = _split8(Ws)
    weff = w_hi.astype(np.float32) + w_lo.astype(np.float32)
    wsum = np.ascontiguousarray(weff.sum(axis=0), dtype=np.float32)
    wsum_2d = np.ascontiguousarray(wsum.reshape(24, 128).T)
    wvs_b = np.broadcast_to(np.clip(wsum[2 * C:], -240, 240).astype(
        ml_dtypes.float8_e4m3), (128, C)).copy()

    # position: per-query max-shift + 1/SCALE scaling, bf16
    m = position.max(axis=1) + M_SHIFT            # [N] per query i
    posp = (position - m[:, None]) / SCALE        # [i, j]

    in_maps = []
    for core in range(8):
        b, s = divmod(core, 2)
        xb = x[b]
        mu = xb.mean(axis=0)
        var = ((xb - mu) ** 2).mean(axis=0)
        rstd = 1.0 / np.sqrt(var + LN_EPS)
        if s == 1:
            xb = np.roll(xb, -MY, axis=1)
            mu = np.roll(mu, -MY)
            rstd = np.roll(rstd, -MY)
            pos_rot = np.roll(posp, -MY, axis=1)
        else:
            pos_rot = posp
        xr = xb * rstd[None, :]
        x_hi, x_lo = _split8(xr)
        nmr = np.clip(-mu * rstd, -240, 240).astype(ml_dtypes.float8_e4m3)
        nmr_b = np.broadcast_to(nmr, (128, N)).copy()
        nmr_col = np.ascontiguousarray(
            (-mu * rstd).reshape(NJT, 128).T, dtype=np.float32)
        pos_t = np.ascontiguousarray(
            pos_rot[s * MY:(s + 1) * MY, :].T).astype(ml_dtypes.bfloat16)
        in_maps.append({
            "x_hi": x_hi, "x_lo": x_lo,
            "w_hi": w_hi, "w_lo": w_lo,
            "nmr_b": nmr_b, "nmr_col": nmr_col,
            "wsum": wsum_2d, "wvs_b": wvs_b,
            "bias": np.ascontiguousarray(bp.reshape(24, 128).T),
            "pos_t": pos_t,
        })
    return in_maps


def kernel(x, position, ln_gamma, ln_beta, W_qkv, b_qkv):
    in_maps = prep_in_maps(x, position, ln_gamma, ln_beta, W_qkv, b_qkv)
    bp = in_maps[0]["bias"]  # [128, 24]: cols 0:16 are q,k; 16:24 are v
    nc = _get_nc(bool(np.abs(bp[:, :16]).max() > 0),
                 bool(np.abs(bp[:, 16:]).max() > 0))
    res = run_bass_kernel_spmd(nc, in_maps, core_ids=list(range(8)))
    out = np.empty((B, C, N), dtype=np.float32)
    for core in range(8):
        b, s = divmod(core, 2)
        out[b, :, s * MY:(s + 1) * MY] = res.results[core]["out"].astype(np.float32).T
    return out
